# revision 1
# baseline (speedup 1.0000x reference)
"""Transformer block (pre-LN attention + MLP) for B=8, T=1024, C=768, H=12.

Sharding: pure data-parallel — one batch element per NeuronCore, identical
SPMD program on cores 0-7, no collectives.

Per-core dataflow (activations kept on-chip end to end):
  x (resident, loaded once; x1 written in place) -> LN1 -> PE-transpose
    -> zt1 bf16 [C, T] -> QKV (w1 f32r stationary): qT bf16 / kT f32r
    head-major, v bf16 natural (+ones col)
    -> attention: S^T = K Q^T (kT f32r stationary x qT bf16 moving, causal
       block-skip), exp on ACT -> per-pair eS bf16 tiles, staircase causal
       masks on DVE, AV (vnat bf16 stationary x eS bf16 moving) -> ctx^T
       + row-sums, normalize via DVE reciprocal + gpsimd bcast -> ctxT bf16
    -> attn_proj (ctxT stationary x wp bf16 moving) + residual in place
    -> LN2 -> zt2 f32r -> FC (w2 f32r x zt2 f32r, pure; gelu fused on ACT)
    -> gT f32r -> MLP proj (gT stationary x wm f32r moving, pure) -> y

Emission order interleaves j=0 heads -> attnproj/LN2 of t0-3 -> j=1 heads
so the scheduler can fill exp-wait PE bubbles with projection work.
Moving-operand bf16 removes the f32r <256-col rate penalty in attention;
pure-f32r pairs elsewhere avoid Ldweights sequencer overhead on the big
GEMM paths.
"""

import threading
from contextlib import ExitStack

import numpy as np

import concourse.bass as bass
from concourse import bacc
import concourse.mybir as mybir
import concourse.tile as tile
from concourse.bass_utils import run_bass_kernel_spmd
from concourse.masks import make_identity


# ---------------------------------------------------------------------------

B, T, C, H, HD, F, P = 8, 1024, 768, 12, 64, 3072, 128
NT = T // P        # 8  t-chunks
NCC = C // P       # 6  c-chunks
NFH = F // 2 // P  # 12 f-chunks per half
EPS = 1e-5
ATTN_SCALE = 1.0 / 8.0

F32 = mybir.dt.float32
MM_DT = mybir.dt.float32r
BF16 = mybir.dt.bfloat16


def build_module(dbg: bool = False) -> bass.Bass:
    nc = bacc.Bacc()

    x_h = nc.dram_tensor("x", [T, C], BF16, kind="ExternalInput")
    w1_h = nc.dram_tensor("w1", [C, 3 * C], BF16, kind="ExternalInput")
    b1_h = nc.dram_tensor("b1", [3 * C], F32, kind="ExternalInput")
    b1v_h = nc.dram_tensor("b1v", [C], BF16, kind="ExternalInput")
    wp_h = nc.dram_tensor("wp", [C, C], BF16, kind="ExternalInput")
    bp_h = nc.dram_tensor("bp", [C], F32, kind="ExternalInput")
    w2_h = nc.dram_tensor("w2", [C, F], F32, kind="ExternalInput")
    b2_h = nc.dram_tensor("b2", [F], F32, kind="ExternalInput")
    wm_h = nc.dram_tensor("wm", [F, C], F32, kind="ExternalInput")
    bm_h = nc.dram_tensor("bm", [C], F32, kind="ExternalInput")
    y_h = nc.dram_tensor("y", [T, C], BF16, kind="ExternalOutput")

    def bcast_ap(ap1d, n_part=P):
        return bass.AP(
            tensor=ap1d.tensor, offset=ap1d.offset, ap=[[0, n_part], *ap1d.ap]
        )

    xr = x_h.ap().rearrange("(tc p) c -> p tc c", p=P)
    yr = y_h.ap().rearrange("(tc p) c -> p tc c", p=P)
    w1r = w1_h.ap().rearrange("(kc p) d -> p kc d", p=P)
    w2r = w2_h.ap().rearrange("(kc p) f -> p kc f", p=P)
    wmr = wm_h.ap().rearrange("(fc p) c -> p fc c", p=P)

    with tile.TileContext(nc) as tc, ExitStack() as top:
        consts = top.enter_context(tc.tile_pool(name="consts", bufs=1))

        ident = consts.tile([P, P], BF16)
        make_identity(nc, ident[:])
        eps_t = consts.tile([P, 1], F32)
        nc.vector.memset(eps_t[:], EPS)
        ones_col = consts.tile([P, 1], F32)
        nc.vector.memset(ones_col[:], 1.0)
        scratch = consts.tile([P, 1], F32)
        # prefetch the Sqrt act-table while the first x chunk is in flight
        nc.scalar.activation(
            out=scratch[:], in_=eps_t[:],
            func=mybir.ActivationFunctionType.Sqrt, scale=1.0,
        )
        b1qk = consts.tile([P, 2 * C // P], F32)
        b2c = consts.tile([P, F // P], F32)

        # ---- long-lived pools (stack allocator: open order = close order
        # reversed) --------------------------------------------------------
        xp_ctx = ExitStack()
        xp = xp_ctx.enter_context(tc.tile_pool(name="xp", bufs=1))
        # x chunks, overwritten in place by x1 = x + attn_out at stage 3
        x1 = [
            xp.tile([P, C], BF16, tag=f"x_{i}", name=f"xc_{i}")
            for i in range(NT)
        ]

        mlpw_ctx = ExitStack()
        w2p = mlpw_ctx.enter_context(tc.tile_pool(name="w2p", bufs=6))
        lnm = mlpw_ctx.enter_context(tc.tile_pool(name="lnm", bufs=1))
        sta = mlpw_ctx.enter_context(tc.tile_pool(name="sta", bufs=3))
        mvs2 = lnm.tile([P, NT, 2], F32)
        rstds2 = lnm.tile([P, NT], F32)
        zt2p = mlpw_ctx.enter_context(tc.tile_pool(name="zt2p", bufs=1))
        zt2 = zt2p.tile([P, NCC, T], MM_DT)

        ctp_ctx = ExitStack()
        ctp = ctp_ctx.enter_context(tc.tile_pool(name="ctp", bufs=1))
        ctxT = [
            ctp.tile([P, T], BF16, tag=f"ctxT{cc}", name=f"ctxT_{cc}")
            for cc in range(NCC)
        ]

        wpp_ctx = ExitStack()
        wpp = wpp_ctx.enter_context(tc.tile_pool(name="wpp", bufs=1))
        wp_sb = wpp.tile([P, NCC, C], BF16)
        bp_t = wpp.tile([1, C], MM_DT)
        ones1 = wpp.tile([1, P], F32)

        qkv_ctx = ExitStack()
        qkp = qkv_ctx.enter_context(tc.tile_pool(name="qkp", bufs=1))
        vp = qkv_ctx.enter_context(tc.tile_pool(name="vp", bufs=1))
        qT_t = qkp.tile([P, NCC, T], BF16)
        kT_t = qkp.tile([P, NCC, T], BF16)
        vnat = vp.tile([P, NT, H, HD + 1], BF16)
        b1v_b = vp.tile([P, C], BF16)

        zt1_ctx = ExitStack()
        zt1p = zt1_ctx.enter_context(tc.tile_pool(name="zt1p", bufs=1))
        zt1 = zt1p.tile([P, NCC, T], BF16)

        w1_ctx = ExitStack()
        w1p = w1_ctx.enter_context(tc.tile_pool(name="w1p", bufs=12))

        # ---- startup DMA interleave: x chunks + w1 q/k passes -------------
        w1q = [
            w1p.tile([P, C], BF16, tag="w1s", name=f"w1s_q_{kc}")
            for kc in range(NCC)
        ]
        w1k = [
            w1p.tile([P, C], BF16, tag="w1s", name=f"w1s_k_{kc}")
            for kc in range(NCC)
        ]

        # tiny bias/const loads first so nothing downstream queues behind a
        # buffer-rotation stall
        nc.sync.dma_start(
            b1qk[:], b1_h.ap()[0 : 2 * C].rearrange("(dc p) -> p dc", p=P)
        )
        nc.sync.dma_start(b2c[:], b2_h.ap().rearrange("(fc p) -> p fc", p=P))
        nc.sync.dma_start(bp_t[0:1, :], bp_h.ap().bitcast(MM_DT))
        nc.vector.memset(ones1[:], 1.0)
        nc.sync.dma_start(x1[0][:], xr[:, 0, :])
        nc.sync.dma_start(x1[1][:], xr[:, 1, :])
        for kc in range(NCC):
            if kc + 2 < NT:
                nc.sync.dma_start(x1[kc + 2][:], xr[:, kc + 2, :])
            nc.sync.dma_start(w1q[kc][:], w1r[:, kc, 0:C])
        for kc in range(NCC):
            nc.sync.dma_start(w1k[kc][:], w1r[:, kc, C : 2 * C])
        nc.gpsimd.dma_start(b1v_b[:], bcast_ap(b1v_h.ap()))

        # v weights reuse the q slots once the q-pass is done
        def load_w1v():
            tiles = []
            for kc in range(NCC):
                w1s = w1p.tile([P, C], BF16, tag="w1s", name=f"w1s_v_{kc}")
                nc.sync.dma_start(w1s[:], w1r[:, kc, 2 * C : 3 * C])
                tiles.append(w1s)
            return tiles

        # ---- LN + transpose helper (copies split DVE/ACT) -----------------
        last_rstd = [None]

        def layernorm_transpose(get_src, zt_tile, zpool, spool, pst):
            for tci in range(NT):
                xt = get_src(tci)
                stats = spool.tile([P, 2, 6], F32, tag="stats")
                for s in range(2):
                    nc.vector.bn_stats(
                        out=stats[:, s, :], in_=xt[:, s * 384 : (s + 1) * 384]
                    )
                mv = spool.tile([P, 2], F32, tag="mv")
                nc.vector.bn_aggr(out=mv[:], in_=stats[:])
                rstd = spool.tile([P, 1], F32, tag="rstd")
                nc.scalar.activation(
                    out=rstd[:],
                    in_=mv[:, 1:2],
                    func=mybir.ActivationFunctionType.Sqrt,
                    bias=eps_t[:],
                    scale=1.0,
                )
                nc.vector.reciprocal(out=rstd[:], in_=rstd[:])
                last_rstd[0] = rstd
                z = zpool.tile([P, C], BF16, tag="z")
                for hh in range(2):
                    nc.vector.tensor_scalar(
                        out=z[:, hh * 384 : (hh + 1) * 384],
                        in0=xt[:, hh * 384 : (hh + 1) * 384],
                        scalar1=mv[:, 0:1],
                        scalar2=rstd[:],
                        op0=mybir.AluOpType.subtract,
                        op1=mybir.AluOpType.mult,
                    )
                for cc in range(NCC):
                    pt = pst.tile([P, P], BF16, tag="pt")
                    nc.tensor.transpose(
                        pt[:], z[:, cc * P : (cc + 1) * P], ident[:]
                    )
                    dst = zt_tile[:, cc, tci * P : (tci + 1) * P]
                    if cc % 3 == 0:
                        nc.vector.tensor_copy(out=dst, in_=pt[:])
                    else:
                        nc.scalar.copy(out=dst, in_=pt[:])

        # ---- stage 1a: LN1 (x already resident) ---------------------------
        with (
            tc.tile_pool(name="ln_z", bufs=3) as _zp,
            tc.tile_pool(name="ln_s", bufs=3) as _sp,
            tc.tile_pool(name="ln_pt", bufs=6, space="PSUM") as _pp,
        ):
            layernorm_transpose(lambda t: x1[t][:], zt1, _zp, _sp, _pp)


        def load_w2_half(half):
            tiles = []
            for kc in range(NCC):
                w2s = w2p.tile(
                    [P, F // 2], MM_DT, tag="w2s", name=f"w2s_{half}_{kc}"
                )
                nc.sync.dma_start(
                    w2s[:],
                    w2r[:, kc, half * (F // 2) : (half + 1) * (F // 2)].bitcast(
                        MM_DT
                    ),
                )
                tiles.append(w2s)
            return tiles


        # v weights before the mlp prefetches: the sync DMA queue is FIFO, so
        # anything QKV needs must precede DMAs that can stall on buffer reuse
        w1v = load_w1v()

        # ---- attention pools that span the QKV merge ----------------------
        lnm_is_placeholder = None  # (marker)

        cm_ctx = ExitStack()
        att_ctx = ExitStack()
        esp = att_ctx.enter_context(tc.tile_pool(name="esp", bufs=19))
        rcp = att_ctx.enter_context(tc.tile_pool(name="rcp", bufs=2))
        bcp = att_ctx.enter_context(tc.tile_pool(name="bcp", bufs=2))
        # cmask lives in consts (cmp pool removed to fit SBUF)
        ps_s = att_ctx.enter_context(
            tc.tile_pool(name="ps_s", bufs=2, space="PSUM")
        )
        cmask = consts.tile([P, P], F32)
        nc.vector.memset(cmask[:], 1.0)
        nc.gpsimd.affine_select(
            out=cmask[:],
            in_=cmask[:],
            compare_op=mybir.AluOpType.is_ge,
            fill=0.0,
            base=0,
            pattern=[[1, P]],
            channel_multiplier=-1,
        )

        last_eS = [None]

        def emit_scores(j, h):
            """Scores + exp for head h, q-half j. Returns the eS pair tiles."""
            row = (h % 2) * HD
            qT_h = qT_t[row : row + HD, h // 2, :]
            kT_h = kT_t[row : row + HD, h // 2, :]
            eS = []
            # full (non-diagonal) k-chunks, exp batched in pairs
            for half in range(2 * j):
                psS = ps_s.tile([P, 2, 512], F32, tag="psS")
                eSt = esp.tile([P, 2, 512], BF16, tag="eS")
                for mi in range(2):
                    m = half * 2 + mi
                    nc.tensor.matmul(
                        psS[:, mi, :],
                        (kT_h[:, m * P : (m + 1) * P]),
                        (qT_h[:, j * 512 : (j + 1) * 512]),
                        start=True,
                        stop=True,
                    )
                nc.scalar.activation(
                    out=eSt[:],
                    in_=psS[:],
                    func=mybir.ActivationFunctionType.Exp,
                    scale=ATTN_SCALE,
                )
                eS.append(eSt)
            # diagonal-crossing k-chunks: only live columns q >= r
            for di in range(2):
                psS = ps_s.tile([P, 2, 512], F32, tag="psS")
                eSt = esp.tile([P, 2, 512], BF16, tag="eS")
                for mi in range(2):
                    m = 4 * j + di * 2 + mi
                    r = m * P - 512 * j
                    nc.tensor.matmul(
                        psS[:, mi, r:512],
                        (kT_h[:, m * P : (m + 1) * P]),
                        (qT_h[:, j * 512 + r : (j + 1) * 512]),
                        start=True,
                        stop=True,
                    )
                for mi in range(2):
                    m = 4 * j + di * 2 + mi
                    r = m * P - 512 * j
                    nc.scalar.activation(
                        out=eSt[:, mi, r:512],
                        in_=psS[:, mi, r:512],
                        func=mybir.ActivationFunctionType.Exp,
                        scale=ATTN_SCALE,
                    )
                    nc.gpsimd.affine_select(
                        out=eSt[:, mi, r : r + P],
                        in_=eSt[:, mi, r : r + P],
                        compare_op=mybir.AluOpType.is_ge,
                        fill=0.0,
                        base=0,
                        pattern=[[1, P]],
                        channel_multiplier=-1,
                    )
                eS.append(eSt)
            last_eS[0] = eS[-1]
            return eS

        def emit_av(j, h, eS):
            row = (h % 2) * HD
            nm = 4 * (j + 1)
            psC = ps_c.tile([HD + 1, 512], F32, tag="psC")
            for m in range(nm):
                r = max(0, m * P - 512 * j)
                nc.tensor.matmul(
                    psC[:, r:512],
                    (vnat[:, m, h, :]),
                    (eS[m // 2][:, m % 2, r:512]),
                    start=(m == 0),
                    stop=(m == nm - 1),
                )
            recip = rcp.tile([1, 512], BF16, tag="recip")
            with nc.allow_low_precision(reason="softmax scale in bf16"):
                nc.vector.reciprocal(out=recip[:], in_=psC[HD : HD + 1, :])
            bc = bcp.tile([HD, 512], BF16, tag="bc")
            nc.gpsimd.partition_broadcast(bc[:], recip[:])
            nc.vector.tensor_mul(
                out=ctxT[h // 2][row : row + HD, j * 512 : (j + 1) * 512],
                in0=psC[0:HD, :],
                in1=bc[:],
            )

        # ---- stage 1b: QKV merged with j=0 scores/exp ----------------------
        with (
            tc.tile_pool(name="ps_qk", bufs=2, space="PSUM") as ps_qk,
            tc.tile_pool(name="ps_v", bufs=1, space="PSUM") as ps_v,
        ):
            def emit_qk(qk, dcl, split_first=False):
                w1s = w1q if qk == 0 else w1k
                dc = qk * NCC + dcl
                dst_t = qT_t if qk == 0 else kT_t
                for j in range(2):
                    ps = ps_qk.tile([P, 512], F32, tag="psqk")
                    if split_first and j == 0:
                        # two 256-col chains so the first matmuls only need
                        # the first two LN1 chunks transposed
                        for half in range(2):
                            for kc in range(NCC):
                                nc.tensor.matmul(
                                    ps[:, half * 256 : (half + 1) * 256],
                                    (w1s[kc][:, dcl * P : (dcl + 1) * P]),
                                    (zt1[:, kc,
                                         half * 256 : (half + 1) * 256]),
                                    start=(kc == 0),
                                    stop=(kc == NCC - 1),
                                )
                    else:
                        for kc in range(NCC):
                            nc.tensor.matmul(
                                ps[:],
                                (w1s[kc][:, dcl * P : (dcl + 1) * P]),
                                (zt1[:, kc, j * 512 : (j + 1) * 512]),
                                start=(kc == 0),
                                stop=(kc == NCC - 1),
                            )
                    nc.vector.tensor_scalar_add(
                        out=dst_t[:, dcl, j * 512 : (j + 1) * 512],
                        in0=ps[:],
                        scalar1=b1qk[:, dc : dc + 1],
                    )

            def emit_v(w1v, tci):
                psv5 = ps_v.tile([P, 512], F32, tag="psv5")
                psv2 = ps_v.tile([P, 256], F32, tag="psv2")
                for kc in range(NCC):
                    nc.tensor.matmul(
                        psv5[:],
                        (zt1[:, kc, tci * P : (tci + 1) * P]),
                        (w1v[kc][:, 0:512]),
                        start=(kc == 0),
                        stop=(kc == NCC - 1),
                    )
                    nc.tensor.matmul(
                        psv2[:],
                        (zt1[:, kc, tci * P : (tci + 1) * P]),
                        (w1v[kc][:, 512:768]),
                        start=(kc == 0),
                        stop=(kc == NCC - 1),
                    )
                nc.vector.tensor_add(
                    out=vnat[:, tci, 0:8, 0:HD],
                    in0=psv5[:].rearrange("p (h d) -> p h d", h=8),
                    in1=b1v_b[:, 0:512].rearrange("p (h d) -> p h d", h=8),
                )
                nc.vector.tensor_add(
                    out=vnat[:, tci, 8:12, 0:HD],
                    in0=psv2[:].rearrange("p (h d) -> p h d", h=4),
                    in1=b1v_b[:, 512:768].rearrange("p (h d) -> p h d", h=4),
                )
                nc.vector.tensor_copy(
                    out=vnat[:, tci, :, HD : HD + 1].rearrange(
                        "p h one -> p (h one)"
                    ),
                    in_=ones_col[:].to_broadcast((P, H)),
                )

            emit_qk(0, 0)
            emit_qk(1, 0)
            for tci in range(4):
                emit_v(w1v, tci)
            # Exp table prefetch (input dep on LN1 t7 rstd orders it after
            # the last LN1 sqrt)
            nc.scalar.activation(
                out=scratch[:], in_=last_rstd[0][:],
                func=mybir.ActivationFunctionType.Exp, scale=1.0,
            )
            eS0 = {}
            eS0[0] = emit_scores(0, 0)
            eS0[1] = emit_scores(0, 1)
            for dcl in range(1, NCC):
                emit_qk(0, dcl)
                emit_qk(1, dcl)
                if dcl <= 4:
                    emit_v(w1v, 3 + dcl)
                eS0[2 * dcl] = emit_scores(0, 2 * dcl)
                eS0[2 * dcl + 1] = emit_scores(0, 2 * dcl + 1)

        # wp + h0 FC weights stream during attention (wm streams during FC)
        for kc in range(NCC):
            nc.sync.dma_start(
                wp_sb[:, kc, :],
                wp_h.ap().rearrange("(kc p) c -> p kc c", p=P)[:, kc, :],
            )
        w2h0 = load_w2_half(0)

        # ---- stage 2: AV(j0), j=1 heads, attn_proj, LN2 stats --------------
        att2_ctx = ExitStack()
        ps_c = att2_ctx.enter_context(
            tc.tile_pool(name="ps_c", bufs=2, space="PSUM")
        )
        ps_ap = att2_ctx.enter_context(
            tc.tile_pool(name="ps_ap", bufs=1, space="PSUM")
        )
        ap_pool = [ps_ap]

        def emit_attnproj(tci):
            ps = ap_pool[0].tile([P, C], F32, tag="psap")
            for kc in range(NCC):
                nc.tensor.matmul(
                    ps[:, 0:512],
                    (ctxT[kc][:, tci * P : (tci + 1) * P]),
                    (wp_sb[:, kc, 0:512]),
                    start=(kc == 0),
                    stop=False,
                )
                nc.tensor.matmul(
                    ps[:, 512:768],
                    (ctxT[kc][:, tci * P : (tci + 1) * P]),
                    (wp_sb[:, kc, 512:768]),
                    start=(kc == 0),
                    stop=False,
                )
            # bp folded in as a K=1 rank-1 update (ones x bp)
            nc.tensor.matmul(
                ps[:, 0:512], ones1[:].bitcast(MM_DT), bp_t[0:1, 0:512],
                start=False, stop=True,
            )
            nc.tensor.matmul(
                ps[:, 512:768], ones1[:].bitcast(MM_DT), bp_t[0:1, 512:768],
                start=False, stop=True,
            )
            # x1 = x + attn_out + bp, overwriting the resident x chunk
            nc.vector.tensor_add(out=x1[tci][:], in0=ps[:], in1=x1[tci][:])
            # LN2 stats for this chunk (sqrt batched later: Exp stays the
            # loaded ACT table during attention)
            stats = sta.tile([P, 2, 6], F32, tag="stats")
            for s in range(2):
                nc.vector.bn_stats(
                    out=stats[:, s, :],
                    in_=x1[tci][:, s * 384 : (s + 1) * 384],
                )
            nc.vector.bn_aggr(out=mvs2[:, tci, :], in_=stats[:])

        eS1 = {}
        for h in range(H):
            emit_av(0, h, eS0[h])
            if h % 2 == 1:
                hh1 = (h - 1) // 2
                eS1[hh1] = emit_scores(1, hh1)
        for h in range(6):
            eS1[h + 6] = emit_scores(1, h + 6)
            emit_av(1, h, eS1[h])
            if h >= 1 and h <= 4:
                emit_attnproj(h - 1)
        for h in range(6, H):
            emit_av(1, h, eS1[h])
        # Sqrt table prefetch ordered after the last exp
        nc.scalar.activation(
            out=scratch[:], in_=last_eS[0][:, 1, 511:512],
            func=mybir.ActivationFunctionType.Sqrt, scale=1.0,
        )

        att2_ctx.close()
        att_ctx.close()

        cm_ctx.close()
        w1_ctx.close()
        zt1_ctx.close()
        qkv_ctx.close()

        # ---- attn_proj t4-7 overlapped with LN2 first half ----------------
        mlp_ps_ctx = ExitStack()
        ps_fc = mlp_ps_ctx.enter_context(
            tc.tile_pool(name="ps_fc", bufs=2, space="PSUM")
        )
        attB_ctx = ExitStack()
        ps_apB = attB_ctx.enter_context(
            tc.tile_pool(name="ps_apB", bufs=2, space="PSUM")
        )
        ln2z = attB_ctx.enter_context(tc.tile_pool(name="ln2z", bufs=3))
        ln2pt = attB_ctx.enter_context(
            tc.tile_pool(name="ln2pt", bufs=2, space="PSUM")
        )
        ap_pool[0] = ps_apB

        def emit_ln2(tci):
            z = ln2z.tile([P, C], BF16, tag="z")
            for hh in range(2):
                nc.vector.tensor_scalar(
                    out=z[:, hh * 384 : (hh + 1) * 384],
                    in0=x1[tci][:, hh * 384 : (hh + 1) * 384],
                    scalar1=mvs2[:, tci, 0:1],
                    scalar2=rstds2[:, tci : tci + 1],
                    op0=mybir.AluOpType.subtract,
                    op1=mybir.AluOpType.mult,
                )
            for cc in range(NCC):
                pt = ln2pt.tile([P, P], BF16, tag="pt")
                nc.tensor.transpose(
                    pt[:], z[:, cc * P : (cc + 1) * P], ident[:]
                )
                dst = zt2[:, cc, tci * P : (tci + 1) * P]
                if cc % 3 == 0:
                    nc.vector.tensor_copy(out=dst, in_=pt[:])
                else:
                    nc.scalar.copy(out=dst, in_=pt[:])

        # first-half rstd (all exps are done: single Sqrt table load)
        nc.scalar.activation(
            out=rstds2[:, 0:4],
            in_=mvs2[:, 0:4, 1],
            func=mybir.ActivationFunctionType.Sqrt,
            bias=eps_t[:],
            scale=1.0,
        )
        nc.vector.reciprocal(out=rstds2[:, 0:4], in_=rstds2[:, 0:4])
        for tci in range(4, NT):
            emit_ln2(tci - 4)
            emit_attnproj(tci)
        nc.scalar.activation(
            out=rstds2[:, 4:8],
            in_=mvs2[:, 4:8, 1],
            func=mybir.ActivationFunctionType.Sqrt,
            bias=eps_t[:],
            scale=1.0,
        )
        nc.vector.reciprocal(out=rstds2[:, 4:8], in_=rstds2[:, 4:8])
        for tci in range(4, NT):
            emit_ln2(tci)

        attB_ctx.close()

        # ---- stage 4+5: MLP ------------------------------------------------
        wm_ctx = ExitStack()
        wmp = wm_ctx.enter_context(tc.tile_pool(name="wmp", bufs=12))

        def load_wm_half(half):
            tiles = []
            for kc in range(NFH):
                wms = wmp.tile(
                    [P, C], MM_DT, tag="wms", name=f"wms_{half}_{kc}"
                )
                nc.sync.dma_start(
                    wms[:], wmr[:, half * NFH + kc, :].bitcast(MM_DT)
                )
                tiles.append(wms)
            return tiles

        wmh0 = load_wm_half(0)
        # prefetch the Gelu table while the FC matmuls accumulate
        nc.scalar.activation(
            out=scratch[:], in_=rstds2[:, 4:5],
            func=mybir.ActivationFunctionType.Gelu_apprx_tanh, scale=1.0,
        )
        with (
            tc.tile_pool(name="mlpc", bufs=1) as mlpc,
            tc.tile_pool(name="gtp", bufs=1) as gtp,
            tc.tile_pool(name="ps_mlp", bufs=3, space="PSUM") as ps_mlp,
        ):
            bm_b = mlpc.tile([P, C], F32)
            nc.gpsimd.dma_start(bm_b[:], bcast_ap(bm_h.ap()))

            for half in range(2):
                w2s = w2h0 if half == 0 else load_w2_half(1)
                wms = wmh0 if half == 0 else load_wm_half(1)
                gT = gtp.tile([P, NFH, T], MM_DT, tag="gT", name=f"gT_{half}")
                for j in range(2):
                    for mf in range(NFH):
                        fc_glob = half * NFH + mf
                        ps = ps_fc.tile([P, 512], F32, tag="psfc")
                        for kc in range(NCC):
                            nc.tensor.matmul(
                                ps[:],
                                (w2s[kc][:, mf * P : (mf + 1) * P]),
                                (zt2[:, kc, j * 512 : (j + 1) * 512]),
                                start=(kc == 0),
                                stop=(kc == NCC - 1),
                            )
                        nc.scalar.activation(
                            out=gT[:, mf, j * 512 : (j + 1) * 512],
                            in_=ps[:],
                            func=mybir.ActivationFunctionType.Gelu_apprx_tanh,
                            bias=b2c[:, fc_glob : fc_glob + 1],
                            scale=1.0,
                        )
                for grp in ((0,), (1,), (2,), (3,), (4,), (5,), (6,), (7,)):
                    pss = {}
                    for tci in grp:
                        psm = ps_mlp.tile(
                            [P, C], F32, tag="psmlp", name=f"psm_{half}_{tci}"
                        )
                        pss[tci] = psm
                    for kc in range(NFH):
                        last = kc == NFH - 1
                        for tci in grp:
                            nc.tensor.matmul(
                                pss[tci][:, 0:512],
                                (gT[:, kc, tci * P : (tci + 1) * P]),
                                (wms[kc][:, 0:512]),
                                start=(kc == 0),
                                stop=last,
                            )
                            nc.tensor.matmul(
                                pss[tci][:, 512:768],
                                (gT[:, kc, tci * P : (tci + 1) * P]),
                                (wms[kc][:, 512:768]),
                                start=(kc == 0),
                                stop=last,
                            )
                    for tci in grp:
                        nc.vector.tensor_add(
                            out=x1[tci][:], in0=x1[tci][:], in1=pss[tci][:]
                        )
                        if half == 0:
                            # bm on DVE: the MLP window is PE-bound, DVE idle
                            nc.vector.tensor_add(
                                out=x1[tci][:], in0=x1[tci][:], in1=bm_b[:]
                            )
                        else:
                            nc.sync.dma_start(yr[:, tci, :], x1[tci][:])

        wm_ctx.close()
        mlp_ps_ctx.close()
        wpp_ctx.close()
        ctp_ctx.close()
        mlpw_ctx.close()
        xp_ctx.close()

    nc.compile()
    return nc


# ---------------------------------------------------------------------------
# host wrapper
# ---------------------------------------------------------------------------

_module_cache: dict = {}
_module_lock = threading.Lock()


def _get_module(dbg: bool = False) -> bass.Bass:
    with _module_lock:
        if dbg not in _module_cache:
            _module_cache[dbg] = build_module(dbg)
        return _module_cache[dbg]


def _fold_inputs(
    x, ln1_scale, ln1_bias, w_qkv, b_qkv, w_attn_proj, b_attn_proj,
    ln2_scale, ln2_bias, w_fc, b_fc, w_mlp_proj, b_mlp_proj,
):
    import ml_dtypes

    f32 = np.float32
    bf16 = ml_dtypes.bfloat16
    w1 = (ln1_scale[:, None].astype(np.float64) * w_qkv.astype(np.float64)).astype(f32)
    b1 = (b_qkv.astype(np.float64) + ln1_bias.astype(np.float64) @ w_qkv.astype(np.float64)).astype(f32)
    w2 = (ln2_scale[:, None].astype(np.float64) * w_fc.astype(np.float64)).astype(f32)
    b2 = (b_fc.astype(np.float64) + ln2_bias.astype(np.float64) @ w_fc.astype(np.float64)).astype(f32)
    shared = {
        "w1": np.ascontiguousarray(w1.astype(bf16)),
        "b1": np.ascontiguousarray(b1),
        "b1v": np.ascontiguousarray(b1[2 * C : 3 * C].astype(bf16)),
        "wp": np.ascontiguousarray(w_attn_proj.astype(f32).astype(bf16)),
        "bp": np.ascontiguousarray(b_attn_proj.astype(f32)),
        "w2": np.ascontiguousarray(w2),
        "b2": np.ascontiguousarray(b2),
        "wm": np.ascontiguousarray(w_mlp_proj.astype(f32)),
        "bm": np.ascontiguousarray(b_mlp_proj.astype(f32)),
    }
    return [
        {"x": np.ascontiguousarray(x[b].astype(f32).astype(bf16)), **shared} for b in range(B)
    ]


def run(inputs: dict, dbg: bool = False, **spmd_kwargs):
    """Run on 8 cores; returns BassKernelResults."""
    args = {k: np.asarray(v) for k, v in inputs.items()}
    in_maps = _fold_inputs(
        args["x"], args["ln1_scale"], args["ln1_bias"], args["w_qkv"],
        args["b_qkv"], args["w_attn_proj"], args["b_attn_proj"],
        args["ln2_scale"], args["ln2_bias"], args["w_fc"], args["b_fc"],
        args["w_mlp_proj"], args["b_mlp_proj"],
    )
    nc = _get_module(dbg)
    res = run_bass_kernel_spmd(nc, in_maps, core_ids=list(range(B)), **spmd_kwargs)
    return res


def kernel(**inputs) -> np.ndarray:
    res = run(inputs)
    return np.stack([res.results[b]["y"] for b in range(B)], axis=0).astype(
        np.float32
    )


if __name__ == "__main__":
    build_module(dbg=False)
    print("module built OK")



# revision 61
# speedup vs baseline: 1.1072x; 1.1072x over previous
"""Transformer block (pre-LN attention + MLP) for B=8, T=1024, C=768, H=12.

Sharding: pure data-parallel — one batch element per NeuronCore, identical
SPMD program on cores 0-7, no collectives.

Per-core dataflow (activations kept on-chip end to end):
  x (resident, loaded once; x1 written in place) -> LN1 -> PE-transpose
    -> zt1 fp8 [C, T] -> QKV fp8 DoubleRow (w1 fp8 stationary): qT/kT bf16
    head-major, v fp8 natural (+ones col)
    -> attention: S^T = K Q^T (kT bf16 stationary x qT bf16 moving, causal
       block-skip), exp on ACT (bias -1.5, cancelled by normalization) ->
       per-pair eS fp8 tiles, staircase causal masks on DVE (widened on the
       second-of-pair tile so the pair's dead region is zeroed for free),
       AV fp8 DoubleRow (vnat fp8 stationary x eS fp8 pairs moving) -> ctx^T
       + row-sums, normalize via DVE reciprocal + gpsimd bcast -> ctxT fp8
    -> attn_proj fp8 DoubleRow (ctxT stationary x wp fp8 moving) + residual
    -> LN2 -> zt2 f32r -> FC (w2 f32r x zt2 f32r, pure; gelu fused on ACT)
    -> gT f32r -> MLP proj (gT stationary x wm f32r moving, pure) -> y

fp8e4 DoubleRow (2 k-subtiles per matmul at 0.5 cycles/row) quarters the
PE time of the K-deep attention GEMMs; scores stay bf16 (K=64 per head
cannot pair, and bf16 costs the same as fp8 there while keeping q/k exact).
"""

import threading
from contextlib import ExitStack

import numpy as np

import concourse.bass as bass
from concourse import bacc
import concourse.mybir as mybir
import concourse.tile as tile
from concourse.bass_utils import run_bass_kernel_spmd
from concourse.masks import make_identity


# ---------------------------------------------------------------------------

B, T, C, H, HD, F, P = 8, 1024, 768, 12, 64, 3072, 128
NT = T // P        # 8  t-chunks
NCC = C // P       # 6  c-chunks
NFH = F // 2 // P  # 12 f-chunks per half
EPS = 1e-5
# weights are pre-scaled x16 on the host so their hi/lo fp8e4 splits stay
# clear of the e4m3 denormal floor; q,k each carry x16 so the score scale
# absorbs 1/256, and wp carries the 1/16 for the x16 in v/ctx.
WSCALE = 16.0
ATTN_SCALE = 1.0 / (8.0 * WSCALE * WSCALE)
EXP_BIAS = -2.75   # exp(S*scale - 2.75): keeps eS under fp8e4 max (240)
                   # even for ~8-sigma scores; a per-row-constant shift
                   # cancels in the softmax ratio.

F32 = mybir.dt.float32
MM_DT = mybir.dt.float32r
BF16 = mybir.dt.bfloat16
FP8 = mybir.dt.float8e4
DR = mybir.MatmulPerfMode.DoubleRow


def build_module(dbg: bool = False) -> bass.Bass:
    nc = bacc.Bacc()

    x_h = nc.dram_tensor("x", [T, C], BF16, kind="ExternalInput")
    w1h_h = nc.dram_tensor("w1h", [C, 3 * C], FP8, kind="ExternalInput")
    w1l_h = nc.dram_tensor("w1l", [C, 3 * C], FP8, kind="ExternalInput")
    b1_h = nc.dram_tensor("b1", [3 * C], F32, kind="ExternalInput")
    b1v_h = nc.dram_tensor("b1v", [C], BF16, kind="ExternalInput")
    wpb_h = nc.dram_tensor("wpb", [C, C], BF16, kind="ExternalInput")
    bp_h = nc.dram_tensor("bp", [C], F32, kind="ExternalInput")
    w2h_h = nc.dram_tensor("w2h", [C, F], FP8, kind="ExternalInput")
    w2l_h = nc.dram_tensor("w2l", [C, F], FP8, kind="ExternalInput")
    b2_h = nc.dram_tensor("b2", [F], F32, kind="ExternalInput")
    wmh_h = nc.dram_tensor("wmh", [F, C], FP8, kind="ExternalInput")
    wml_h = nc.dram_tensor("wml", [F, C], FP8, kind="ExternalInput")
    bm_h = nc.dram_tensor("bm", [C], F32, kind="ExternalInput")
    y_h = nc.dram_tensor("y", [T, C], BF16, kind="ExternalOutput")

    def bcast_ap(ap1d, n_part=P):
        return bass.AP(
            tensor=ap1d.tensor, offset=ap1d.offset, ap=[[0, n_part], *ap1d.ap]
        )

    xr = x_h.ap().rearrange("(tc p) c -> p tc c", p=P)
    yr = y_h.ap().rearrange("(tc p) c -> p tc c", p=P)
    w1hr = w1h_h.ap().rearrange("(kc p) d -> p kc d", p=P)
    w1lr = w1l_h.ap().rearrange("(kc p) d -> p kc d", p=P)
    w2hr = w2h_h.ap().rearrange("(kc p) f -> p kc f", p=P)
    w2lr = w2l_h.ap().rearrange("(kc p) f -> p kc f", p=P)
    wmhr = wmh_h.ap().rearrange("(fc p) c -> p fc c", p=P)
    wmlr = wml_h.ap().rearrange("(fc p) c -> p fc c", p=P)

    with tile.TileContext(nc) as tc, ExitStack() as top:
        consts = top.enter_context(tc.tile_pool(name="consts", bufs=1))

        ident = consts.tile([P, P], BF16)
        make_identity(nc, ident[:])
        eps_t = consts.tile([P, 1], F32)
        nc.vector.memset(eps_t[:], EPS)
        negone = consts.tile([P, 1], F32)
        nc.vector.memset(negone[:], EXP_BIAS)
        ones_col = consts.tile([P, 1], F32)
        nc.vector.memset(ones_col[:], 1.0)
        # K=1 broadcast / zeroing helpers for the PE
        ones64 = consts.tile([1, HD], BF16)
        nc.vector.memset(ones64[:], 1.0)
        # dead-region fill: ones-col x (-1e6) row makes exp underflow to 0
        zrow = consts.tile([1, 512], BF16)
        nc.vector.memset(zrow[:], -1.0e6)
        zcol = consts.tile([1, P], BF16)
        nc.vector.memset(zcol[:], 1.0)
        scratch = consts.tile([P, 1], F32)
        # prefetch the Sqrt act-table while the first x chunk is in flight
        nc.scalar.activation(
            out=scratch[:], in_=eps_t[:],
            func=mybir.ActivationFunctionType.Sqrt, scale=1.0,
        )
        b1qk = consts.tile([P, 2 * C // P], F32)
        b2c = consts.tile([P, F // P], F32)

        # ---- long-lived pools (stack allocator: open order = close order
        # reversed) --------------------------------------------------------
        xp_ctx = ExitStack()
        xp = xp_ctx.enter_context(tc.tile_pool(name="xp", bufs=1))
        # x chunks, overwritten in place by x1 = x + attn_out at stage 3
        x1 = [
            xp.tile([P, C], BF16, tag=f"x_{i}", name=f"xc_{i}")
            for i in range(NT)
        ]

        mlpw_ctx = ExitStack()
        w2p = mlpw_ctx.enter_context(tc.tile_pool(name="w2p", bufs=4))
        lnm = mlpw_ctx.enter_context(tc.tile_pool(name="lnm", bufs=1))
        sta = mlpw_ctx.enter_context(tc.tile_pool(name="sta", bufs=3))
        mvs2 = lnm.tile([P, NT, 2], F32)
        rstds2 = lnm.tile([P, NT], F32)
        zt2p = mlpw_ctx.enter_context(tc.tile_pool(name="zt2p", bufs=1))
        zt2h = zt2p.tile([P, NCC, T], FP8)
        zt2l = zt2p.tile([P, NCC, T], FP8)

        ctp_ctx = ExitStack()
        ctp = ctp_ctx.enter_context(tc.tile_pool(name="ctp", bufs=1))
        ctxT = ctp.tile([P, NCC, T], BF16)

        wpp_ctx = ExitStack()
        wpp = wpp_ctx.enter_context(tc.tile_pool(name="wpp", bufs=1))
        wp_b = wpp.tile([P, NCC, C], BF16)
        bp_t = wpp.tile([1, C], MM_DT)
        ones1 = wpp.tile([1, P], F32)

        qkv_ctx = ExitStack()
        qkp = qkv_ctx.enter_context(tc.tile_pool(name="qkp", bufs=1))
        vp = qkv_ctx.enter_context(tc.tile_pool(name="vp", bufs=1))
        qT_t = qkp.tile([P, NCC, T], BF16)
        kT_t = qkp.tile([P, NCC, T], BF16)
        # v in natural layout, heads at stride HD+1 (ones col for row-sums);
        # t-chunk stride padded to a 16B multiple (DoubleRow ldweights
        # requires pair-dim stride % 16 == 0)
        VSTR = 784  # >= H * (HD + 1) = 780, multiple of 16
        vnat = vp.tile([P, NT, VSTR], FP8)
        # bf16 v for t-chunk 0 (feeds the bf16 early-row attention path)
        v0c = vp.tile([P, H * (HD + 1)], BF16)
        b1v_b = vp.tile([P, C], BF16)

        zt1_ctx = ExitStack()
        zt1p = zt1_ctx.enter_context(tc.tile_pool(name="zt1p", bufs=1))
        zt1h = zt1p.tile([P, NCC, T], FP8)
        zt1l = zt1p.tile([P, NCC, T], FP8)

        w1_ctx = ExitStack()
        w1p = w1_ctx.enter_context(tc.tile_pool(name="w1p", bufs=1))
        w1qh = w1p.tile([P, NCC, C], FP8)
        w1ql = w1p.tile([P, NCC, C], FP8)
        w1kh = w1p.tile([P, NCC, C], FP8)
        w1kl = w1p.tile([P, NCC, C], FP8)
        w1vh = w1p.tile([P, NCC, C], FP8)
        w1vl = w1p.tile([P, NCC, C], FP8)

        # ---- startup DMA interleave: x chunks + w1 q/k passes -------------
        # tiny bias/const loads first so nothing downstream queues behind a
        # buffer-rotation stall
        nc.sync.dma_start(
            b1qk[:], b1_h.ap()[0 : 2 * C].rearrange("(dc p) -> p dc", p=P)
        )
        nc.sync.dma_start(b2c[:], b2_h.ap().rearrange("(fc p) -> p fc", p=P))
        nc.sync.dma_start(bp_t[0:1, :], bp_h.ap().bitcast(MM_DT))
        nc.vector.memset(ones1[:], 1.0)
        nc.sync.dma_start(x1[0][:], xr[:, 0, :])
        nc.sync.dma_start(x1[1][:], xr[:, 1, :])
        for kc in range(NCC):
            if kc + 2 < NT:
                nc.sync.dma_start(x1[kc + 2][:], xr[:, kc + 2, :])
            nc.sync.dma_start(w1qh[:, kc, :], w1hr[:, kc, 0:C])
            nc.sync.dma_start(w1ql[:, kc, :], w1lr[:, kc, 0:C])
        for kc in range(NCC):
            nc.sync.dma_start(w1kh[:, kc, :], w1hr[:, kc, C : 2 * C])
            nc.sync.dma_start(w1kl[:, kc, :], w1lr[:, kc, C : 2 * C])
        nc.gpsimd.dma_start(b1v_b[:], bcast_ap(b1v_h.ap()))

        def load_w1v():
            for kc in range(NCC):
                nc.sync.dma_start(w1vh[:, kc, :], w1hr[:, kc, 2 * C : 3 * C])
                nc.sync.dma_start(w1vl[:, kc, :], w1lr[:, kc, 2 * C : 3 * C])

        # ---- LN + transpose helper (batched copies, hi/lo peel) -----------
        last_rstd = [None]

        def layernorm_transpose(get_src, zth, ztl, zpool, spool, pst):
            for tci in range(NT):
                xt = get_src(tci)
                stats = spool.tile([P, 2, 6], F32, tag="stats")
                for s in range(2):
                    nc.vector.bn_stats(
                        out=stats[:, s, :], in_=xt[:, s * 384 : (s + 1) * 384]
                    )
                mv = spool.tile([P, 2], F32, tag="mv")
                nc.vector.bn_aggr(out=mv[:], in_=stats[:])
                rstd = spool.tile([P, 1], F32, tag="rstd")
                nc.scalar.activation(
                    out=rstd[:],
                    in_=mv[:, 1:2],
                    func=mybir.ActivationFunctionType.Sqrt,
                    bias=eps_t[:],
                    scale=1.0,
                )
                nc.vector.reciprocal(out=rstd[:], in_=rstd[:])
                last_rstd[0] = rstd
                z = zpool.tile([P, C], BF16, tag="z")
                for hh in range(2):
                    nc.vector.tensor_scalar(
                        out=z[:, hh * 384 : (hh + 1) * 384],
                        in0=xt[:, hh * 384 : (hh + 1) * 384],
                        scalar1=mv[:, 0:1],
                        scalar2=rstd[:],
                        op0=mybir.AluOpType.subtract,
                        op1=mybir.AluOpType.mult,
                    )
                pt6 = pst.tile([P, NCC, P], BF16, tag="pt")
                for cc in range(NCC):
                    nc.tensor.transpose(
                        pt6[:, cc, :], z[:, cc * P : (cc + 1) * P], ident[:]
                    )
                tsl = slice(tci * P, (tci + 1) * P)
                nc.scalar.copy(out=zth[:, :, tsl], in_=pt6[:])
                if ztl is not None:
                    nc.vector.tensor_tensor(
                        out=ztl[:, :, tsl], in0=pt6[:], in1=zth[:, :, tsl],
                        op=mybir.AluOpType.subtract,
                    )

        # ---- stage 1a: LN1 (x already resident) ---------------------------
        with (
            tc.tile_pool(name="ln_z", bufs=3) as _zp,
            tc.tile_pool(name="ln_s", bufs=3) as _sp,
            tc.tile_pool(name="ln_pt", bufs=3, space="PSUM") as _pp,
        ):
            layernorm_transpose(lambda t: x1[t][:], zt1h, zt1l, _zp, _sp, _pp)


        def load_w2_half(half):
            w2ht = w2p.tile([P, NCC, F // 2], FP8, tag="w2s",
                            name=f"w2h_{half}")
            w2lt = w2p.tile([P, NCC, F // 2], FP8, tag="w2s",
                            name=f"w2l_{half}")
            fs = slice(half * (F // 2), (half + 1) * (F // 2))
            for kc in range(NCC):
                nc.sync.dma_start(w2ht[:, kc, :], w2hr[:, kc, fs])
                nc.sync.dma_start(w2lt[:, kc, :], w2lr[:, kc, fs])
            return w2ht, w2lt


        # v weights before the mlp prefetches: the sync DMA queue is FIFO, so
        # anything QKV needs must precede DMAs that can stall on buffer reuse
        load_w1v()

        # ---- attention pools that span the QKV merge ----------------------
        cm_ctx = ExitStack()
        att_ctx = ExitStack()
        esp = att_ctx.enter_context(tc.tile_pool(name="esp", bufs=26))
        rcp = att_ctx.enter_context(tc.tile_pool(name="rcp", bufs=2))
        bcp = att_ctx.enter_context(tc.tile_pool(name="bcp", bufs=2))
        cep = att_ctx.enter_context(tc.tile_pool(name="cep", bufs=3))
        ps_s = att_ctx.enter_context(
            tc.tile_pool(name="ps_s", bufs=2, space="PSUM")
        )

        last_eS = [None]

        def emit_scores(j, h):
            """Scores + exp for head h, q-half j. Returns the eS pair tiles."""
            row = (h % 2) * HD
            qT_h = qT_t[row : row + HD, h // 2, :]
            kT_h = kT_t[row : row + HD, h // 2, :]
            eS = []
            # full (non-diagonal) k-chunks, exp batched in pairs
            for half in range(2 * j):
                psS = ps_s.tile([P, 2, 512], F32, tag="psS")
                eSt = esp.tile([P, 2, 512], FP8, tag="eS")
                for mi in range(2):
                    m = half * 2 + mi
                    nc.tensor.matmul(
                        psS[:, mi, :],
                        (kT_h[:, m * P : (m + 1) * P]),
                        (qT_h[:, j * 512 : (j + 1) * 512]),
                        start=True,
                        stop=True,
                    )
                nc.scalar.activation(
                    out=eSt[:],
                    in_=psS[:],
                    func=mybir.ActivationFunctionType.Exp,
                    bias=negone[:],
                    scale=ATTN_SCALE,
                )
                eS.append(eSt)
            # diagonal-crossing k-chunk pairs: the pair is exp'd in ONE op
            # over [rp:512]; the second tile's dead psum block [rp:r) is
            # zeroed on the PE (K=1 x zeros) so exp sees defined data, and
            # its widened staircase select zeroes the eS dead region.
            for di in range(2):
                rp = max(0, (4 * j + di * 2) * P - 512 * j)
                psS = ps_s.tile([P, 2, 512], F32, tag="psS")
                eSt = esp.tile([P, 2, 512], FP8, tag="eS")
                for mi in range(2):
                    m = 4 * j + di * 2 + mi
                    r = m * P - 512 * j
                    if r > rp:
                        nc.tensor.matmul(
                            psS[:, mi, rp:r], zcol[:], zrow[:, 0 : r - rp],
                            start=True, stop=True,
                        )
                    nc.tensor.matmul(
                        psS[:, mi, r:512],
                        (kT_h[:, m * P : (m + 1) * P]),
                        (qT_h[:, j * 512 + r : (j + 1) * 512]),
                        start=True,
                        stop=True,
                    )
                nc.scalar.activation(
                    out=eSt[:, :, rp:512],
                    in_=psS[:, :, rp:512],
                    func=mybir.ActivationFunctionType.Exp,
                    bias=negone[:],
                    scale=ATTN_SCALE,
                )
                for mi in range(2):
                    m = 4 * j + di * 2 + mi
                    r = m * P - 512 * j
                    nc.gpsimd.affine_select(
                        out=eSt[:, mi, r : r + P],
                        in_=eSt[:, mi, r : r + P],
                        compare_op=mybir.AluOpType.is_ge,
                        fill=0.0,
                        base=0,
                        pattern=[[1, P]],
                        channel_multiplier=-1,
                    )
                eS.append(eSt)
            last_eS[0] = eS[-1]
            return eS

        def emit_av(j, h, eS):
            row = (h % 2) * HD
            np_ = 2 * (j + 1)
            psC = ps_c.tile([HD + 1, 512], F32, tag="psC")
            for pi in range(np_):
                r = max(0, 2 * pi * P - 512 * j)
                nc.tensor.matmul(
                    psC[:, r:512],
                    (vnat[:, 2 * pi : 2 * pi + 2,
                          h * (HD + 1) : h * (HD + 1) + HD + 1]),
                    (eS[pi][:, :, r:512]),
                    start=(pi == 0),
                    stop=(pi == np_ - 1),
                    perf_mode=DR,
                )
            recip = rcp.tile([1, 512], BF16, tag="recip")
            with nc.allow_low_precision(reason="softmax scale in bf16"):
                nc.vector.reciprocal(out=recip[:], in_=psC[HD : HD + 1, :])
            bc = bcp.tile([HD, 512], BF16, tag="bc")
            nc.gpsimd.partition_broadcast(bc[:], recip[:])
            nc.vector.tensor_mul(
                out=ctxT[row : row + HD, h // 2, j * 512 : (j + 1) * 512],
                in0=psC[0:HD, :],
                in1=bc[:],
            )

        # ---- stage 1b: QKV merged with j=0 scores/exp ----------------------
        with (
            tc.tile_pool(name="ps_qk", bufs=2, space="PSUM") as ps_qk,
            tc.tile_pool(name="ps_v", bufs=1, space="PSUM") as ps_v,
        ):
            def emit_qk(qk, dcl):
                w1s_h = w1qh if qk == 0 else w1kh
                w1s_l = w1ql if qk == 0 else w1kl
                dc = qk * NCC + dcl
                dst_t = qT_t if qk == 0 else kT_t
                dsl = slice(dcl * P, (dcl + 1) * P)
                for j in range(2):
                    jsl = slice(j * 512, (j + 1) * 512)
                    ps = ps_qk.tile([P, 512], F32, tag="psqk")
                    m = 0
                    for ws, zs in (
                        (w1s_h, zt1h), (w1s_h, zt1l), (w1s_l, zt1h),
                    ):
                        for kp in range(NCC // 2):
                            nc.tensor.matmul(
                                ps[:],
                                (ws[:, 2 * kp : 2 * kp + 2, dsl]),
                                (zs[:, 2 * kp : 2 * kp + 2, jsl]),
                                start=(m == 0),
                                stop=(m == 8),
                                perf_mode=DR,
                            )
                            m += 1
                    nc.vector.tensor_scalar_add(
                        out=dst_t[:, dcl, jsl],
                        in0=ps[:],
                        scalar1=b1qk[:, dc : dc + 1],
                    )

            def emit_v_psum(tci, combos):
                psv5 = ps_v.tile([P, 512], F32, tag="psv5")
                psv2 = ps_v.tile([P, 256], F32, tag="psv2")
                n = len(combos) * (NCC // 2)
                m = 0
                for zs, ws in combos:
                    for kp in range(NCC // 2):
                        zsl = zs[:, 2 * kp : 2 * kp + 2,
                                 tci * P : (tci + 1) * P]
                        nc.tensor.matmul(
                            psv5[:], zsl,
                            (ws[:, 2 * kp : 2 * kp + 2, 0:512]),
                            start=(m == 0), stop=(m == n - 1), perf_mode=DR,
                        )
                        nc.tensor.matmul(
                            psv2[:], zsl,
                            (ws[:, 2 * kp : 2 * kp + 2, 512:768]),
                            start=(m == 0), stop=(m == n - 1), perf_mode=DR,
                        )
                        m += 1
                return psv5, psv2

            def write_v(vh, psv5, psv2):
                nc.vector.tensor_add(
                    out=vh[:, 0:8, 0:HD],
                    in0=psv5[:].rearrange("p (h d) -> p h d", h=8),
                    in1=b1v_b[:, 0:512].rearrange("p (h d) -> p h d", h=8),
                )
                nc.vector.tensor_add(
                    out=vh[:, 8:12, 0:HD],
                    in0=psv2[:].rearrange("p (h d) -> p h d", h=4),
                    in1=b1v_b[:, 512:768].rearrange("p (h d) -> p h d", h=4),
                )
                nc.vector.tensor_copy(
                    out=vh[:, :, HD : HD + 1].rearrange(
                        "p h one -> p (h one)"
                    ),
                    in_=ones_col[:].to_broadcast((P, H)),
                )

            def emit_v(tci):
                psv5, psv2 = emit_v_psum(tci, ((zt1h, w1vh),))
                vh = vnat[:, tci, 0 : H * (HD + 1)].rearrange(
                    "p (h e) -> p h e", h=H
                )
                write_v(vh, psv5, psv2)

            def emit_v0c():
                # bf16-grade v for t-chunk 0 (hi/lo corrected)
                psv5, psv2 = emit_v_psum(
                    0, ((zt1h, w1vh), (zt1l, w1vh), (zt1h, w1vl))
                )
                vh = v0c[:].rearrange("p (h e) -> p h e", h=H)
                write_v(vh, psv5, psv2)

            emit_qk(0, 0)
            emit_qk(1, 0)
            for tci in range(4):
                emit_v(tci)
            # Exp table prefetch (input dep on LN1 t7 rstd orders it after
            # the last LN1 sqrt)
            nc.scalar.activation(
                out=scratch[:], in_=last_rstd[0][:],
                func=mybir.ActivationFunctionType.Exp, scale=1.0,
            )
            eS0 = {}
            eS0[0] = emit_scores(0, 0)
            eS0[1] = emit_scores(0, 1)
            for dcl in range(1, NCC):
                emit_qk(0, dcl)
                emit_qk(1, dcl)
                if dcl <= 4:
                    emit_v(3 + dcl)
                eS0[2 * dcl] = emit_scores(0, 2 * dcl)
                eS0[2 * dcl + 1] = emit_scores(0, 2 * dcl + 1)
            emit_v0c()

        # wp + h0 FC weights stream during attention (wm streams during FC)
        for kc in range(NCC):
            nc.sync.dma_start(
                wp_b[:, kc, :],
                wpb_h.ap().rearrange("(kc p) c -> p kc c", p=P)[:, kc, :],
            )
        w2h0 = load_w2_half(0)

        # ---- stage 2: AV(j0), j=1 heads, attn_proj, LN2 stats --------------
        att2_ctx = ExitStack()
        ps_c = att2_ctx.enter_context(
            tc.tile_pool(name="ps_c", bufs=2, space="PSUM")
        )
        ps_ap = att2_ctx.enter_context(
            tc.tile_pool(name="ps_ap", bufs=1, space="PSUM")
        )
        ap_pool = [ps_ap]

        def emit_corr_pair(hp):
            """bf16 attention for q-rows [0:128), head pair (2hp, 2hp+1)."""
            psS = ps_s.tile([P, 2, 512], F32, tag="psS")
            eS0c = cep.tile([P, 2, P], BF16, tag="eS0c")
            for mi in range(2):
                h = 2 * hp + mi
                row = (h % 2) * HD
                nc.tensor.matmul(
                    psS[:, mi, 0:P],
                    (kT_t[row : row + HD, hp, 0:P]),
                    (qT_t[row : row + HD, hp, 0:P]),
                    start=True,
                    stop=True,
                )
            nc.scalar.activation(
                out=eS0c[:],
                in_=psS[:, :, 0:P],
                func=mybir.ActivationFunctionType.Exp,
                bias=negone[:],
                scale=ATTN_SCALE,
            )
            for mi in range(2):
                h = 2 * hp + mi
                row = (h % 2) * HD
                nc.gpsimd.affine_select(
                    out=eS0c[:, mi, :],
                    in_=eS0c[:, mi, :],
                    compare_op=mybir.AluOpType.is_ge,
                    fill=0.0,
                    base=0,
                    pattern=[[1, P]],
                    channel_multiplier=-1,
                )
                psC0 = ps_c.tile([HD + 1, 512], F32, tag="psC")
                nc.tensor.matmul(
                    psC0[:, 0:P],
                    (v0c[:, h * (HD + 1) : (h + 1) * (HD + 1)]),
                    (eS0c[:, mi, :]),
                    start=True,
                    stop=True,
                )
                recip = rcp.tile([1, 512], BF16, tag="recip")
                with nc.allow_low_precision(reason="softmax scale in bf16"):
                    nc.vector.reciprocal(
                        out=recip[:, 0:P], in_=psC0[HD : HD + 1, 0:P]
                    )
                bc = bcp.tile([HD, 512], BF16, tag="bc")
                nc.gpsimd.partition_broadcast(bc[:, 0:P], recip[:, 0:P])
                # overwrites the fp8-grade main-path result for q in [0:128)
                nc.vector.tensor_mul(
                    out=ctxT[row : row + HD, hp, 0:P],
                    in0=psC0[0:HD, 0:P],
                    in1=bc[:, 0:P],
                )

        def emit_attnproj(tci):
            ps = ap_pool[0].tile([P, C], F32, tag="psap")
            for cc in range(NCC):
                nc.tensor.matmul(
                    ps[:, 0:512],
                    (ctxT[:, cc, tci * P : (tci + 1) * P]),
                    (wp_b[:, cc, 0:512]),
                    start=(cc == 0), stop=False,
                )
                nc.tensor.matmul(
                    ps[:, 512:768],
                    (ctxT[:, cc, tci * P : (tci + 1) * P]),
                    (wp_b[:, cc, 512:768]),
                    start=(cc == 0), stop=False,
                )
            # bp folded in as a K=1 rank-1 update (ones x bp)
            nc.tensor.matmul(
                ps[:, 0:512], ones1[:].bitcast(MM_DT), bp_t[0:1, 0:512],
                start=False, stop=True,
            )
            nc.tensor.matmul(
                ps[:, 512:768], ones1[:].bitcast(MM_DT), bp_t[0:1, 512:768],
                start=False, stop=True,
            )
            # x1 = x + attn_out + bp, overwriting the resident x chunk
            nc.vector.tensor_add(out=x1[tci][:], in0=ps[:], in1=x1[tci][:])
            # LN2 stats for this chunk (sqrt batched later: Exp stays the
            # loaded ACT table during attention)
            stats = sta.tile([P, 2, 6], F32, tag="stats")
            for s in range(2):
                nc.vector.bn_stats(
                    out=stats[:, s, :],
                    in_=x1[tci][:, s * 384 : (s + 1) * 384],
                )
            nc.vector.bn_aggr(out=mvs2[:, tci, :], in_=stats[:])

        eS1 = {}
        for h in range(H):
            emit_av(0, h, eS0[h])
            if h % 2 == 1:
                hh1 = (h - 1) // 2
                eS1[hh1] = emit_scores(1, hh1)
                # after both heads' j0 AV: bf16 redo of q-rows [0:128)
                emit_corr_pair(hh1)
        for h in range(6):
            eS1[h + 6] = emit_scores(1, h + 6)
            emit_av(1, h, eS1[h])
            if h >= 1 and h <= 4:
                emit_attnproj(h - 1)
        for h in range(6, H):
            emit_av(1, h, eS1[h])
        # Sqrt table prefetch ordered after the last exp
        nc.scalar.activation(
            out=scratch[:], in_=last_eS[0][:, 1, 511:512],
            func=mybir.ActivationFunctionType.Sqrt, scale=1.0,
        )

        att2_ctx.close()
        att_ctx.close()

        cm_ctx.close()
        w1_ctx.close()
        zt1_ctx.close()
        qkv_ctx.close()

        # ---- attn_proj t4-7 overlapped with LN2 first half ----------------
        mlp_ps_ctx = ExitStack()
        ps_fc = mlp_ps_ctx.enter_context(
            tc.tile_pool(name="ps_fc", bufs=2, space="PSUM")
        )
        attB_ctx = ExitStack()
        ps_apB = attB_ctx.enter_context(
            tc.tile_pool(name="ps_apB", bufs=2, space="PSUM")
        )
        ln2z = attB_ctx.enter_context(tc.tile_pool(name="ln2z", bufs=3))
        ln2pt = attB_ctx.enter_context(
            tc.tile_pool(name="ln2pt", bufs=2, space="PSUM")
        )
        ap_pool[0] = ps_apB

        def emit_ln2(tci):
            z = ln2z.tile([P, C], BF16, tag="z")
            for hh in range(2):
                nc.vector.tensor_scalar(
                    out=z[:, hh * 384 : (hh + 1) * 384],
                    in0=x1[tci][:, hh * 384 : (hh + 1) * 384],
                    scalar1=mvs2[:, tci, 0:1],
                    scalar2=rstds2[:, tci : tci + 1],
                    op0=mybir.AluOpType.subtract,
                    op1=mybir.AluOpType.mult,
                )
            # transposes batched into one PSUM tile; single strided copies
            # peel z2 into hi (fp8) + lo (residual, fp8) halves
            pt6 = ln2pt.tile([P, NCC, P], BF16, tag="pt")
            for cc in range(NCC):
                nc.tensor.transpose(
                    pt6[:, cc, :], z[:, cc * P : (cc + 1) * P], ident[:]
                )
            tsl = slice(tci * P, (tci + 1) * P)
            nc.scalar.copy(out=zt2h[:, :, tsl], in_=pt6[:])
            nc.vector.tensor_tensor(
                out=zt2l[:, :, tsl], in0=pt6[:], in1=zt2h[:, :, tsl],
                op=mybir.AluOpType.subtract,
            )

        # first-half rstd (all exps are done: single Sqrt table load)
        nc.scalar.activation(
            out=rstds2[:, 0:4],
            in_=mvs2[:, 0:4, 1],
            func=mybir.ActivationFunctionType.Sqrt,
            bias=eps_t[:],
            scale=1.0,
        )
        nc.vector.reciprocal(out=rstds2[:, 0:4], in_=rstds2[:, 0:4])
        for tci in range(4, NT):
            emit_ln2(tci - 4)
            emit_attnproj(tci)
        nc.scalar.activation(
            out=rstds2[:, 4:8],
            in_=mvs2[:, 4:8, 1],
            func=mybir.ActivationFunctionType.Sqrt,
            bias=eps_t[:],
            scale=1.0,
        )
        nc.vector.reciprocal(out=rstds2[:, 4:8], in_=rstds2[:, 4:8])
        for tci in range(4, NT):
            emit_ln2(tci)

        attB_ctx.close()

        # ---- stage 4+5: MLP ------------------------------------------------
        wm_ctx = ExitStack()
        wmp = wm_ctx.enter_context(tc.tile_pool(name="wmp", bufs=4))

        def load_wm_half(half):
            wmht = wmp.tile([P, NFH, C], FP8, tag="wms", name=f"wmh_{half}")
            wmlt = wmp.tile([P, NFH, C], FP8, tag="wms", name=f"wml_{half}")
            for kc in range(NFH):
                nc.sync.dma_start(wmht[:, kc, :], wmhr[:, half * NFH + kc, :])
                nc.sync.dma_start(wmlt[:, kc, :], wmlr[:, half * NFH + kc, :])
            return wmht, wmlt

        wmh0 = load_wm_half(0)
        # prefetch the Gelu table while the FC matmuls accumulate
        nc.scalar.activation(
            out=scratch[:], in_=rstds2[:, 4:5],
            func=mybir.ActivationFunctionType.Gelu_apprx_tanh, scale=1.0,
        )
        with (
            tc.tile_pool(name="mlpc", bufs=1) as mlpc,
            tc.tile_pool(name="gtp", bufs=1) as gtp,
            tc.tile_pool(name="gq", bufs=3) as gqp,
            tc.tile_pool(name="ps_mlp", bufs=3, space="PSUM") as ps_mlp,
        ):
            bm_b = mlpc.tile([P, C], F32)
            nc.gpsimd.dma_start(bm_b[:], bcast_ap(bm_h.ap()))

            for half in range(2):
                w2ht, w2lt = w2h0 if half == 0 else load_w2_half(1)
                wmht, wmlt = wmh0 if half == 0 else load_wm_half(1)
                gTh = gtp.tile([P, NFH, T], FP8, tag="gTh", name=f"gTh_{half}")
                gTl = gtp.tile([P, NFH, T], FP8, tag="gTl", name=f"gTl_{half}")
                for j in range(2):
                    for mf in range(NFH):
                        fc_glob = half * NFH + mf
                        ms = slice(mf * P, (mf + 1) * P)
                        js = slice(j * 512, (j + 1) * 512)
                        ps = ps_fc.tile([P, 512], F32, tag="psfc")
                        n = 0
                        for wt, zt in (
                            (w2ht, zt2h), (w2ht, zt2l), (w2lt, zt2h),
                        ):
                            for kp in range(NCC // 2):
                                nc.tensor.matmul(
                                    ps[:],
                                    wt[:, 2 * kp : 2 * kp + 2, ms],
                                    zt[:, 2 * kp : 2 * kp + 2, js],
                                    start=(n == 0),
                                    stop=(n == 8),
                                    perf_mode=DR,
                                )
                                n += 1
                        # gelu twice on ACT (fp8-hi + bf16 exact); DVE peels
                        # the residual into gTl.  scale 1/16 undoes the w2
                        # host pre-scale.
                        gq = gqp.tile([P, 512], BF16, tag="gq")
                        nc.scalar.activation(
                            out=gTh[:, mf, js],
                            in_=ps[:],
                            func=mybir.ActivationFunctionType.Gelu_apprx_tanh,
                            bias=b2c[:, fc_glob : fc_glob + 1],
                            scale=1.0 / WSCALE,
                        )
                        nc.scalar.activation(
                            out=gq[:],
                            in_=ps[:],
                            func=mybir.ActivationFunctionType.Gelu_apprx_tanh,
                            bias=b2c[:, fc_glob : fc_glob + 1],
                            scale=1.0 / WSCALE,
                        )
                        nc.vector.tensor_tensor(
                            out=gTl[:, mf, js], in0=gq[:], in1=gTh[:, mf, js],
                            op=mybir.AluOpType.subtract,
                        )
                for grp in ((0,), (1,), (2,), (3,), (4,), (5,), (6,), (7,)):
                    pss = {}
                    for tci in grp:
                        psm = ps_mlp.tile(
                            [P, C], F32, tag="psmlp", name=f"psm_{half}_{tci}"
                        )
                        pss[tci] = psm
                    for tci in grp:
                        tsl = slice(tci * P, (tci + 1) * P)
                        m = 0
                        for gt, wt in (
                            (gTh, wmht), (gTl, wmht), (gTh, wmlt),
                        ):
                            for kp in range(NFH // 2):
                                nc.tensor.matmul(
                                    pss[tci][:, 0:512],
                                    gt[:, 2 * kp : 2 * kp + 2, tsl],
                                    wt[:, 2 * kp : 2 * kp + 2, 0:512],
                                    start=(m == 0),
                                    stop=(m == 17),
                                    perf_mode=DR,
                                )
                                nc.tensor.matmul(
                                    pss[tci][:, 512:768],
                                    gt[:, 2 * kp : 2 * kp + 2, tsl],
                                    wt[:, 2 * kp : 2 * kp + 2, 512:768],
                                    start=(m == 0),
                                    stop=(m == 17),
                                    perf_mode=DR,
                                )
                                m += 1
                    for tci in grp:
                        # fused (psum * 1/16) + x1 on DVE undoes the wm x16
                        # pre-scale while draining PSUM
                        nc.vector.scalar_tensor_tensor(
                            out=x1[tci][:], in0=pss[tci][:],
                            scalar=1.0 / WSCALE, in1=x1[tci][:],
                            op0=mybir.AluOpType.mult,
                            op1=mybir.AluOpType.add,
                        )
                        if half == 0:
                            # bm on DVE: the MLP window is PE-bound, DVE idle
                            nc.vector.tensor_add(
                                out=x1[tci][:], in0=x1[tci][:], in1=bm_b[:]
                            )
                        else:
                            nc.sync.dma_start(yr[:, tci, :], x1[tci][:])

        wm_ctx.close()
        mlp_ps_ctx.close()
        wpp_ctx.close()
        ctp_ctx.close()
        mlpw_ctx.close()
        xp_ctx.close()

    nc.compile()
    return nc


# ---------------------------------------------------------------------------
# host wrapper
# ---------------------------------------------------------------------------

_module_cache: dict = {}
_module_lock = threading.Lock()


def _get_module(dbg: bool = False) -> bass.Bass:
    with _module_lock:
        if dbg not in _module_cache:
            _module_cache[dbg] = build_module(dbg)
        return _module_cache[dbg]


def _fold_inputs(
    x, ln1_scale, ln1_bias, w_qkv, b_qkv, w_attn_proj, b_attn_proj,
    ln2_scale, ln2_bias, w_fc, b_fc, w_mlp_proj, b_mlp_proj,
):
    import ml_dtypes

    f32 = np.float32
    bf16 = ml_dtypes.bfloat16
    fp8 = ml_dtypes.float8_e4m3
    w1 = (ln1_scale[:, None].astype(np.float64) * w_qkv.astype(np.float64)).astype(f32)
    b1 = (b_qkv.astype(np.float64) + ln1_bias.astype(np.float64) @ w_qkv.astype(np.float64)).astype(f32)
    w2 = (ln2_scale[:, None].astype(np.float64) * w_fc.astype(np.float64)).astype(f32)
    b2 = (b_fc.astype(np.float64) + ln2_bias.astype(np.float64) @ w_fc.astype(np.float64)).astype(f32)
    def hilo(w):
        # x16 pre-scale keeps the residual (lo) part of these ~N(0, 1/sqrt
        # (fan_in)) weights clear of the fp8e4 denormal floor
        ws = w.astype(np.float64) * WSCALE
        hi = ws.astype(f32).astype(fp8)
        lo = (ws - hi.astype(np.float64)).astype(f32).astype(fp8)
        return np.ascontiguousarray(hi), np.ascontiguousarray(lo)

    w2h, w2l = hilo(w2)
    wmh, wml = hilo(w_mlp_proj.astype(f32))
    w1h, w1l = hilo(w1)
    shared = {
        "w1h": w1h,
        "w1l": w1l,
        "b1": np.ascontiguousarray(b1 * np.float32(WSCALE)),
        "b1v": np.ascontiguousarray(
            (b1[2 * C : 3 * C] * np.float32(WSCALE)).astype(bf16)
        ),
        "wpb": np.ascontiguousarray(
            (w_attn_proj.astype(f32) / np.float32(WSCALE)).astype(bf16)
        ),
        "bp": np.ascontiguousarray(b_attn_proj.astype(f32)),
        "w2h": w2h,
        "w2l": w2l,
        "b2": np.ascontiguousarray(b2),
        "wmh": wmh,
        "wml": wml,
        "bm": np.ascontiguousarray(b_mlp_proj.astype(f32)),
    }
    return [
        {"x": np.ascontiguousarray(x[b].astype(f32).astype(bf16)), **shared} for b in range(B)
    ]


def run(inputs: dict, dbg: bool = False, **spmd_kwargs):
    """Run on 8 cores; returns BassKernelResults."""
    args = {k: np.asarray(v) for k, v in inputs.items()}
    in_maps = _fold_inputs(
        args["x"], args["ln1_scale"], args["ln1_bias"], args["w_qkv"],
        args["b_qkv"], args["w_attn_proj"], args["b_attn_proj"],
        args["ln2_scale"], args["ln2_bias"], args["w_fc"], args["b_fc"],
        args["w_mlp_proj"], args["b_mlp_proj"],
    )
    nc = _get_module(dbg)
    res = run_bass_kernel_spmd(nc, in_maps, core_ids=list(range(B)), **spmd_kwargs)
    return res


def kernel(**inputs) -> np.ndarray:
    res = run(inputs)
    return np.stack([res.results[b]["y"] for b in range(B)], axis=0).astype(
        np.float32
    )


if __name__ == "__main__":
    build_module(dbg=False)
    print("module built OK")


# revision 62
# speedup vs baseline: 1.1247x; 1.0159x over previous
"""Transformer block (pre-LN attention + MLP) for B=8, T=1024, C=768, H=12.

Sharding: pure data-parallel — one batch element per NeuronCore, identical
SPMD program on cores 0-7, no collectives.

Per-core dataflow (activations kept on-chip end to end):
  x (resident, loaded once; x1 written in place) -> LN1 -> PE-transpose
    -> zt1 fp8 [C, T] -> QKV fp8 DoubleRow (w1 fp8 stationary): qT/kT bf16
    head-major, v fp8 natural (+ones col)
    -> attention: S^T = K Q^T (kT bf16 stationary x qT bf16 moving, causal
       block-skip), exp on ACT (bias -1.5, cancelled by normalization) ->
       per-pair eS fp8 tiles, staircase causal masks on DVE (widened on the
       second-of-pair tile so the pair's dead region is zeroed for free),
       AV fp8 DoubleRow (vnat fp8 stationary x eS fp8 pairs moving) -> ctx^T
       + row-sums, normalize via DVE reciprocal + gpsimd bcast -> ctxT fp8
    -> attn_proj fp8 DoubleRow (ctxT stationary x wp fp8 moving) + residual
    -> LN2 -> zt2 f32r -> FC (w2 f32r x zt2 f32r, pure; gelu fused on ACT)
    -> gT f32r -> MLP proj (gT stationary x wm f32r moving, pure) -> y

fp8e4 DoubleRow (2 k-subtiles per matmul at 0.5 cycles/row) quarters the
PE time of the K-deep attention GEMMs; scores stay bf16 (K=64 per head
cannot pair, and bf16 costs the same as fp8 there while keeping q/k exact).
"""

import threading
from contextlib import ExitStack

import numpy as np

import concourse.bass as bass
from concourse import bacc
import concourse.mybir as mybir
import concourse.tile as tile
from concourse.bass_utils import run_bass_kernel_spmd
from concourse.masks import make_identity


# ---------------------------------------------------------------------------

B, T, C, H, HD, F, P = 8, 1024, 768, 12, 64, 3072, 128
NT = T // P        # 8  t-chunks
NCC = C // P       # 6  c-chunks
NFH = F // 2 // P  # 12 f-chunks per half
EPS = 1e-5
# weights are pre-scaled x16 on the host so their hi/lo fp8e4 splits stay
# clear of the e4m3 denormal floor; q,k each carry x16 so the score scale
# absorbs 1/256, and wp carries the 1/16 for the x16 in v/ctx.
WSCALE = 16.0
ATTN_SCALE = 1.0 / (8.0 * WSCALE * WSCALE)
EXP_BIAS = -2.75   # exp(S*scale - 2.75): keeps eS under fp8e4 max (240)
                   # even for ~8-sigma scores; a per-row-constant shift
                   # cancels in the softmax ratio.

F32 = mybir.dt.float32
MM_DT = mybir.dt.float32r
BF16 = mybir.dt.bfloat16
FP8 = mybir.dt.float8e4
DR = mybir.MatmulPerfMode.DoubleRow


def build_module(dbg: bool = False) -> bass.Bass:
    nc = bacc.Bacc()

    x_h = nc.dram_tensor("x", [T, C], BF16, kind="ExternalInput")
    w1h_h = nc.dram_tensor("w1h", [C, 3 * C], FP8, kind="ExternalInput")
    w1l_h = nc.dram_tensor("w1l", [C, 3 * C], FP8, kind="ExternalInput")
    b1_h = nc.dram_tensor("b1", [3 * C], F32, kind="ExternalInput")
    b1v_h = nc.dram_tensor("b1v", [C], BF16, kind="ExternalInput")
    wpb_h = nc.dram_tensor("wpb", [C, C], BF16, kind="ExternalInput")
    bp_h = nc.dram_tensor("bp", [C], F32, kind="ExternalInput")
    w2h_h = nc.dram_tensor("w2h", [C, F], FP8, kind="ExternalInput")
    w2l_h = nc.dram_tensor("w2l", [C, F], FP8, kind="ExternalInput")
    b2_h = nc.dram_tensor("b2", [F], F32, kind="ExternalInput")
    wmh_h = nc.dram_tensor("wmh", [F, C], FP8, kind="ExternalInput")
    wml_h = nc.dram_tensor("wml", [F, C], FP8, kind="ExternalInput")
    bm_h = nc.dram_tensor("bm", [C], F32, kind="ExternalInput")
    y_h = nc.dram_tensor("y", [T, C], BF16, kind="ExternalOutput")

    def bcast_ap(ap1d, n_part=P):
        return bass.AP(
            tensor=ap1d.tensor, offset=ap1d.offset, ap=[[0, n_part], *ap1d.ap]
        )

    xr = x_h.ap().rearrange("(tc p) c -> p tc c", p=P)
    yr = y_h.ap().rearrange("(tc p) c -> p tc c", p=P)
    w1hr = w1h_h.ap().rearrange("(kc p) d -> p kc d", p=P)
    w1lr = w1l_h.ap().rearrange("(kc p) d -> p kc d", p=P)
    w2hr = w2h_h.ap().rearrange("(kc p) f -> p kc f", p=P)
    w2lr = w2l_h.ap().rearrange("(kc p) f -> p kc f", p=P)
    wmhr = wmh_h.ap().rearrange("(fc p) c -> p fc c", p=P)
    wmlr = wml_h.ap().rearrange("(fc p) c -> p fc c", p=P)

    with tile.TileContext(nc) as tc, ExitStack() as top:
        consts = top.enter_context(tc.tile_pool(name="consts", bufs=1))

        ident = consts.tile([P, P], BF16)
        make_identity(nc, ident[:])
        eps_t = consts.tile([P, 1], F32)
        nc.vector.memset(eps_t[:], EPS)
        negone = consts.tile([P, 1], F32)
        nc.vector.memset(negone[:], EXP_BIAS)
        ones_col = consts.tile([P, 1], F32)
        nc.vector.memset(ones_col[:], 1.0)
        # K=1 broadcast / zeroing helpers for the PE
        ones64 = consts.tile([1, HD], BF16)
        nc.vector.memset(ones64[:], 1.0)
        # dead-region fill: ones-col x (-1e6) row makes exp underflow to 0
        zrow = consts.tile([1, 512], BF16)
        nc.vector.memset(zrow[:], -1.0e6)
        zcol = consts.tile([1, P], BF16)
        nc.vector.memset(zcol[:], 1.0)
        scratch = consts.tile([P, 1], F32)
        # prefetch the Sqrt act-table while the first x chunk is in flight
        nc.scalar.activation(
            out=scratch[:], in_=eps_t[:],
            func=mybir.ActivationFunctionType.Sqrt, scale=1.0,
        )
        b1qk = consts.tile([P, 2 * C // P], F32)
        b2c = consts.tile([P, F // P], F32)

        # ---- long-lived pools (stack allocator: open order = close order
        # reversed) --------------------------------------------------------
        xp_ctx = ExitStack()
        xp = xp_ctx.enter_context(tc.tile_pool(name="xp", bufs=1))
        # x chunks, overwritten in place by x1 = x + attn_out at stage 3
        x1 = [
            xp.tile([P, C], BF16, tag=f"x_{i}", name=f"xc_{i}")
            for i in range(NT)
        ]

        mlpw_ctx = ExitStack()
        w2p = mlpw_ctx.enter_context(tc.tile_pool(name="w2p", bufs=4))
        lnm = mlpw_ctx.enter_context(tc.tile_pool(name="lnm", bufs=1))
        sta = mlpw_ctx.enter_context(tc.tile_pool(name="sta", bufs=3))
        mvs2 = lnm.tile([P, NT, 2], F32)
        rstds2 = lnm.tile([P, NT], F32)
        zt2p = mlpw_ctx.enter_context(tc.tile_pool(name="zt2p", bufs=1))
        zt2h = zt2p.tile([P, NCC, T], FP8)
        zt2l = zt2p.tile([P, NCC, T], FP8)

        ctp_ctx = ExitStack()
        ctp = ctp_ctx.enter_context(tc.tile_pool(name="ctp", bufs=1))
        ctxT = ctp.tile([P, NCC, T], BF16)

        wpp_ctx = ExitStack()
        wpp = wpp_ctx.enter_context(tc.tile_pool(name="wpp", bufs=1))
        wp_b = wpp.tile([P, NCC, C], BF16)
        bp_t = wpp.tile([1, C], MM_DT)
        ones1 = wpp.tile([1, P], F32)

        qkv_ctx = ExitStack()
        qkp = qkv_ctx.enter_context(tc.tile_pool(name="qkp", bufs=1))
        vp = qkv_ctx.enter_context(tc.tile_pool(name="vp", bufs=1))
        qT_t = qkp.tile([P, NCC, T], BF16)
        kT_t = qkp.tile([P, NCC, T], BF16)
        # v in natural layout, heads at stride HD+1 (ones col for row-sums);
        # t-chunk stride padded to a 16B multiple (DoubleRow ldweights
        # requires pair-dim stride % 16 == 0)
        VSTR = 784  # >= H * (HD + 1) = 780, multiple of 16
        vnat = vp.tile([P, NT, VSTR], FP8)
        # bf16 v for t-chunk 0 (feeds the bf16 early-row attention path)
        v0c = vp.tile([P, H * (HD + 1)], BF16)
        b1v_b = vp.tile([P, C], BF16)

        zt1_ctx = ExitStack()
        zt1p = zt1_ctx.enter_context(tc.tile_pool(name="zt1p", bufs=1))
        zt1h = zt1p.tile([P, NCC, T], FP8)
        zt1l = zt1p.tile([P, NCC, T], FP8)

        w1_ctx = ExitStack()
        w1p = w1_ctx.enter_context(tc.tile_pool(name="w1p", bufs=1))
        w1qh = w1p.tile([P, NCC, C], FP8)
        w1ql = w1p.tile([P, NCC, C], FP8)
        w1kh = w1p.tile([P, NCC, C], FP8)
        w1kl = w1p.tile([P, NCC, C], FP8)
        w1vh = w1p.tile([P, NCC, C], FP8)
        w1vl = w1p.tile([P, NCC, C], FP8)

        # ---- startup DMA interleave: x chunks + w1 q/k passes -------------
        # tiny bias/const loads first so nothing downstream queues behind a
        # buffer-rotation stall
        nc.sync.dma_start(
            b1qk[:], b1_h.ap()[0 : 2 * C].rearrange("(dc p) -> p dc", p=P)
        )
        nc.sync.dma_start(b2c[:], b2_h.ap().rearrange("(fc p) -> p fc", p=P))
        nc.sync.dma_start(bp_t[0:1, :], bp_h.ap().bitcast(MM_DT))
        nc.vector.memset(ones1[:], 1.0)
        nc.sync.dma_start(x1[0][:], xr[:, 0, :])
        nc.sync.dma_start(x1[1][:], xr[:, 1, :])
        for kc in range(NCC):
            if kc + 2 < NT:
                nc.sync.dma_start(x1[kc + 2][:], xr[:, kc + 2, :])
            nc.sync.dma_start(w1qh[:, kc, :], w1hr[:, kc, 0:C])
            nc.sync.dma_start(w1ql[:, kc, :], w1lr[:, kc, 0:C])
        for kc in range(NCC):
            nc.sync.dma_start(w1kh[:, kc, :], w1hr[:, kc, C : 2 * C])
            nc.sync.dma_start(w1kl[:, kc, :], w1lr[:, kc, C : 2 * C])
        nc.gpsimd.dma_start(b1v_b[:], bcast_ap(b1v_h.ap()))

        def load_w1v():
            for kc in range(NCC):
                nc.sync.dma_start(w1vh[:, kc, :], w1hr[:, kc, 2 * C : 3 * C])
                nc.sync.dma_start(w1vl[:, kc, :], w1lr[:, kc, 2 * C : 3 * C])

        # ---- LN + transpose helper (batched copies, hi/lo peel) -----------
        last_rstd = [None]

        def layernorm_transpose(get_src, zth, ztl, zpool, spool, pst):
            for tci in range(NT):
                xt = get_src(tci)
                stats = spool.tile([P, 2, 6], F32, tag="stats")
                for s in range(2):
                    nc.vector.bn_stats(
                        out=stats[:, s, :], in_=xt[:, s * 384 : (s + 1) * 384]
                    )
                mv = spool.tile([P, 2], F32, tag="mv")
                nc.vector.bn_aggr(out=mv[:], in_=stats[:])
                rstd = spool.tile([P, 1], F32, tag="rstd")
                nc.scalar.activation(
                    out=rstd[:],
                    in_=mv[:, 1:2],
                    func=mybir.ActivationFunctionType.Sqrt,
                    bias=eps_t[:],
                    scale=1.0,
                )
                nc.vector.reciprocal(out=rstd[:], in_=rstd[:])
                last_rstd[0] = rstd
                z = zpool.tile([P, C], BF16, tag="z")
                for hh in range(2):
                    nc.vector.tensor_scalar(
                        out=z[:, hh * 384 : (hh + 1) * 384],
                        in0=xt[:, hh * 384 : (hh + 1) * 384],
                        scalar1=mv[:, 0:1],
                        scalar2=rstd[:],
                        op0=mybir.AluOpType.subtract,
                        op1=mybir.AluOpType.mult,
                    )
                pt6 = pst.tile([P, NCC, P], BF16, tag="pt")
                for cc in range(NCC):
                    nc.tensor.transpose(
                        pt6[:, cc, :], z[:, cc * P : (cc + 1) * P], ident[:]
                    )
                tsl = slice(tci * P, (tci + 1) * P)
                nc.scalar.copy(out=zth[:, :, tsl], in_=pt6[:])
                if ztl is not None:
                    nc.vector.tensor_tensor(
                        out=ztl[:, :, tsl], in0=pt6[:], in1=zth[:, :, tsl],
                        op=mybir.AluOpType.subtract,
                    )

        # ---- stage 1a: LN1 (x already resident) ---------------------------
        with (
            tc.tile_pool(name="ln_z", bufs=3) as _zp,
            tc.tile_pool(name="ln_s", bufs=3) as _sp,
            tc.tile_pool(name="ln_pt", bufs=3, space="PSUM") as _pp,
        ):
            layernorm_transpose(lambda t: x1[t][:], zt1h, zt1l, _zp, _sp, _pp)


        def load_w2_half(half):
            w2ht = w2p.tile([P, NCC, F // 2], FP8, tag="w2s",
                            name=f"w2h_{half}")
            w2lt = w2p.tile([P, NCC, F // 2], FP8, tag="w2s",
                            name=f"w2l_{half}")
            fs = slice(half * (F // 2), (half + 1) * (F // 2))
            for kc in range(NCC):
                nc.sync.dma_start(w2ht[:, kc, :], w2hr[:, kc, fs])
                nc.sync.dma_start(w2lt[:, kc, :], w2lr[:, kc, fs])
            return w2ht, w2lt


        # v weights before the mlp prefetches: the sync DMA queue is FIFO, so
        # anything QKV needs must precede DMAs that can stall on buffer reuse
        load_w1v()

        # ---- attention pools that span the QKV merge ----------------------
        cm_ctx = ExitStack()
        att_ctx = ExitStack()
        esp = att_ctx.enter_context(tc.tile_pool(name="esp", bufs=26))
        rcp = att_ctx.enter_context(tc.tile_pool(name="rcp", bufs=2))
        bcp = att_ctx.enter_context(tc.tile_pool(name="bcp", bufs=2))
        cep = att_ctx.enter_context(tc.tile_pool(name="cep", bufs=3))
        ps_s = att_ctx.enter_context(
            tc.tile_pool(name="ps_s", bufs=2, space="PSUM")
        )

        last_eS = [None]

        def emit_scores(j, h):
            """Scores + exp for head h, q-half j. Returns the eS pair tiles."""
            row = (h % 2) * HD
            qT_h = qT_t[row : row + HD, h // 2, :]
            kT_h = kT_t[row : row + HD, h // 2, :]
            eS = []
            # full (non-diagonal) k-chunks, exp batched in pairs
            for half in range(2 * j):
                psS = ps_s.tile([P, 2, 512], F32, tag="psS")
                eSt = esp.tile([P, 2, 512], FP8, tag="eS")
                for mi in range(2):
                    m = half * 2 + mi
                    nc.tensor.matmul(
                        psS[:, mi, :],
                        (kT_h[:, m * P : (m + 1) * P]),
                        (qT_h[:, j * 512 : (j + 1) * 512]),
                        start=True,
                        stop=True,
                    )
                nc.scalar.activation(
                    out=eSt[:],
                    in_=psS[:],
                    func=mybir.ActivationFunctionType.Exp,
                    bias=negone[:],
                    scale=ATTN_SCALE,
                )
                eS.append(eSt)
            # diagonal-crossing k-chunk pairs: the pair is exp'd in ONE op
            # over [rp:512]; the second tile's dead psum block [rp:r) is
            # zeroed on the PE (K=1 x zeros) so exp sees defined data, and
            # its widened staircase select zeroes the eS dead region.
            for di in range(2):
                rp = max(P if j == 0 else 0,
                         (4 * j + di * 2) * P - 512 * j)
                psS = ps_s.tile([P, 2, 512], F32, tag="psS")
                eSt = esp.tile([P, 2, 512], FP8, tag="eS")
                for mi in range(2):
                    m = 4 * j + di * 2 + mi
                    r = max(rp, m * P - 512 * j)
                    if r > rp:
                        nc.tensor.matmul(
                            psS[:, mi, rp:r], zcol[:], zrow[:, 0 : r - rp],
                            start=True, stop=True,
                        )
                    nc.tensor.matmul(
                        psS[:, mi, r:512],
                        (kT_h[:, m * P : (m + 1) * P]),
                        (qT_h[:, j * 512 + r : (j + 1) * 512]),
                        start=True,
                        stop=True,
                    )
                nc.scalar.activation(
                    out=eSt[:, :, rp:512],
                    in_=psS[:, :, rp:512],
                    func=mybir.ActivationFunctionType.Exp,
                    bias=negone[:],
                    scale=ATTN_SCALE,
                )
                for mi in range(2):
                    m = 4 * j + di * 2 + mi
                    r = m * P - 512 * j
                    if r < rp:
                        continue  # staircase block lives in the skipped cols
                    nc.gpsimd.affine_select(
                        out=eSt[:, mi, r : r + P],
                        in_=eSt[:, mi, r : r + P],
                        compare_op=mybir.AluOpType.is_ge,
                        fill=0.0,
                        base=0,
                        pattern=[[1, P]],
                        channel_multiplier=-1,
                    )
                eS.append(eSt)
            last_eS[0] = eS[-1]
            return eS

        def emit_av(j, h, eS):
            row = (h % 2) * HD
            np_ = 2 * (j + 1)
            q0 = P if j == 0 else 0  # q-block 0 comes from the bf16 redo
            psC = ps_c.tile([HD + 1, 512], F32, tag="psC")
            for pi in range(np_):
                r = max(q0, 2 * pi * P - 512 * j)
                nc.tensor.matmul(
                    psC[:, r:512],
                    (vnat[:, 2 * pi : 2 * pi + 2,
                          h * (HD + 1) : h * (HD + 1) + HD + 1]),
                    (eS[pi][:, :, r:512]),
                    start=(pi == 0),
                    stop=(pi == np_ - 1),
                    perf_mode=DR,
                )
            recip = rcp.tile([1, 512], BF16, tag="recip")
            with nc.allow_low_precision(reason="softmax scale in bf16"):
                nc.vector.reciprocal(
                    out=recip[:, q0:512], in_=psC[HD : HD + 1, q0:512]
                )
            bc = bcp.tile([HD, 512], BF16, tag="bc")
            nc.gpsimd.partition_broadcast(bc[:, q0:512], recip[:, q0:512])
            nc.vector.tensor_mul(
                out=ctxT[row : row + HD, h // 2,
                         j * 512 + q0 : (j + 1) * 512],
                in0=psC[0:HD, q0:512],
                in1=bc[:, q0:512],
            )

        # ---- stage 1b: QKV merged with j=0 scores/exp ----------------------
        with (
            tc.tile_pool(name="ps_qk", bufs=2, space="PSUM") as ps_qk,
            tc.tile_pool(name="ps_v", bufs=1, space="PSUM") as ps_v,
        ):
            def emit_qk(qk, dcl):
                w1s_h = w1qh if qk == 0 else w1kh
                w1s_l = w1ql if qk == 0 else w1kl
                dc = qk * NCC + dcl
                dst_t = qT_t if qk == 0 else kT_t
                dsl = slice(dcl * P, (dcl + 1) * P)
                for j in range(2):
                    jsl = slice(j * 512, (j + 1) * 512)
                    ps = ps_qk.tile([P, 512], F32, tag="psqk")
                    m = 0
                    for ws, zs in (
                        (w1s_h, zt1h), (w1s_h, zt1l), (w1s_l, zt1h),
                    ):
                        for kp in range(NCC // 2):
                            nc.tensor.matmul(
                                ps[:],
                                (ws[:, 2 * kp : 2 * kp + 2, dsl]),
                                (zs[:, 2 * kp : 2 * kp + 2, jsl]),
                                start=(m == 0),
                                stop=(m == 8),
                                perf_mode=DR,
                            )
                            m += 1
                    nc.vector.tensor_scalar_add(
                        out=dst_t[:, dcl, jsl],
                        in0=ps[:],
                        scalar1=b1qk[:, dc : dc + 1],
                    )

            def emit_v_psum(tci, combos):
                psv5 = ps_v.tile([P, 512], F32, tag="psv5")
                psv2 = ps_v.tile([P, 256], F32, tag="psv2")
                n = len(combos) * (NCC // 2)
                m = 0
                for zs, ws in combos:
                    for kp in range(NCC // 2):
                        zsl = zs[:, 2 * kp : 2 * kp + 2,
                                 tci * P : (tci + 1) * P]
                        nc.tensor.matmul(
                            psv5[:], zsl,
                            (ws[:, 2 * kp : 2 * kp + 2, 0:512]),
                            start=(m == 0), stop=(m == n - 1), perf_mode=DR,
                        )
                        nc.tensor.matmul(
                            psv2[:], zsl,
                            (ws[:, 2 * kp : 2 * kp + 2, 512:768]),
                            start=(m == 0), stop=(m == n - 1), perf_mode=DR,
                        )
                        m += 1
                return psv5, psv2

            def write_v(vh, psv5, psv2):
                nc.vector.tensor_add(
                    out=vh[:, 0:8, 0:HD],
                    in0=psv5[:].rearrange("p (h d) -> p h d", h=8),
                    in1=b1v_b[:, 0:512].rearrange("p (h d) -> p h d", h=8),
                )
                nc.vector.tensor_add(
                    out=vh[:, 8:12, 0:HD],
                    in0=psv2[:].rearrange("p (h d) -> p h d", h=4),
                    in1=b1v_b[:, 512:768].rearrange("p (h d) -> p h d", h=4),
                )
                nc.vector.tensor_copy(
                    out=vh[:, :, HD : HD + 1].rearrange(
                        "p h one -> p (h one)"
                    ),
                    in_=ones_col[:].to_broadcast((P, H)),
                )

            def emit_v(tci):
                psv5, psv2 = emit_v_psum(tci, ((zt1h, w1vh),))
                vh = vnat[:, tci, 0 : H * (HD + 1)].rearrange(
                    "p (h e) -> p h e", h=H
                )
                write_v(vh, psv5, psv2)

            def emit_v0c():
                # bf16-grade v for t-chunk 0 (hi/lo corrected)
                psv5, psv2 = emit_v_psum(
                    0, ((zt1h, w1vh), (zt1l, w1vh), (zt1h, w1vl))
                )
                vh = v0c[:].rearrange("p (h e) -> p h e", h=H)
                write_v(vh, psv5, psv2)

            emit_qk(0, 0)
            emit_qk(1, 0)
            for tci in range(4):
                emit_v(tci)
            # Exp table prefetch (input dep on LN1 t7 rstd orders it after
            # the last LN1 sqrt)
            nc.scalar.activation(
                out=scratch[:], in_=last_rstd[0][:],
                func=mybir.ActivationFunctionType.Exp, scale=1.0,
            )
            eS0 = {}
            eS0[0] = emit_scores(0, 0)
            eS0[1] = emit_scores(0, 1)
            for dcl in range(1, NCC):
                emit_qk(0, dcl)
                emit_qk(1, dcl)
                if dcl <= 4:
                    emit_v(3 + dcl)
                eS0[2 * dcl] = emit_scores(0, 2 * dcl)
                eS0[2 * dcl + 1] = emit_scores(0, 2 * dcl + 1)
            emit_v0c()

        # wp + h0 FC weights stream during attention (wm streams during FC)
        for kc in range(NCC):
            nc.sync.dma_start(
                wp_b[:, kc, :],
                wpb_h.ap().rearrange("(kc p) c -> p kc c", p=P)[:, kc, :],
            )
        w2h0 = load_w2_half(0)

        # ---- stage 2: AV(j0), j=1 heads, attn_proj, LN2 stats --------------
        att2_ctx = ExitStack()
        ps_c = att2_ctx.enter_context(
            tc.tile_pool(name="ps_c", bufs=2, space="PSUM")
        )
        ps_ap = att2_ctx.enter_context(
            tc.tile_pool(name="ps_ap", bufs=1, space="PSUM")
        )
        ap_pool = [ps_ap]

        def emit_corr_pair(hp):
            """bf16 attention for q-rows [0:128), head pair (2hp, 2hp+1)."""
            psS = ps_s.tile([P, 2, 512], F32, tag="psS")
            eS0c = cep.tile([P, 2, P], BF16, tag="eS0c")
            for mi in range(2):
                h = 2 * hp + mi
                row = (h % 2) * HD
                nc.tensor.matmul(
                    psS[:, mi, 0:P],
                    (kT_t[row : row + HD, hp, 0:P]),
                    (qT_t[row : row + HD, hp, 0:P]),
                    start=True,
                    stop=True,
                )
            nc.scalar.activation(
                out=eS0c[:],
                in_=psS[:, :, 0:P],
                func=mybir.ActivationFunctionType.Exp,
                bias=negone[:],
                scale=ATTN_SCALE,
            )
            for mi in range(2):
                h = 2 * hp + mi
                row = (h % 2) * HD
                nc.gpsimd.affine_select(
                    out=eS0c[:, mi, :],
                    in_=eS0c[:, mi, :],
                    compare_op=mybir.AluOpType.is_ge,
                    fill=0.0,
                    base=0,
                    pattern=[[1, P]],
                    channel_multiplier=-1,
                )
                psC0 = ps_c.tile([HD + 1, 512], F32, tag="psC")
                nc.tensor.matmul(
                    psC0[:, 0:P],
                    (v0c[:, h * (HD + 1) : (h + 1) * (HD + 1)]),
                    (eS0c[:, mi, :]),
                    start=True,
                    stop=True,
                )
                recip = rcp.tile([1, 512], BF16, tag="recip")
                with nc.allow_low_precision(reason="softmax scale in bf16"):
                    nc.vector.reciprocal(
                        out=recip[:, 0:P], in_=psC0[HD : HD + 1, 0:P]
                    )
                bc = bcp.tile([HD, 512], BF16, tag="bc")
                nc.gpsimd.partition_broadcast(bc[:, 0:P], recip[:, 0:P])
                # overwrites the fp8-grade main-path result for q in [0:128)
                nc.vector.tensor_mul(
                    out=ctxT[row : row + HD, hp, 0:P],
                    in0=psC0[0:HD, 0:P],
                    in1=bc[:, 0:P],
                )

        def emit_attnproj(tci):
            ps = ap_pool[0].tile([P, C], F32, tag="psap")
            for cc in range(NCC):
                nc.tensor.matmul(
                    ps[:, 0:512],
                    (ctxT[:, cc, tci * P : (tci + 1) * P]),
                    (wp_b[:, cc, 0:512]),
                    start=(cc == 0), stop=False,
                )
                nc.tensor.matmul(
                    ps[:, 512:768],
                    (ctxT[:, cc, tci * P : (tci + 1) * P]),
                    (wp_b[:, cc, 512:768]),
                    start=(cc == 0), stop=False,
                )
            # bp folded in as a K=1 rank-1 update (ones x bp)
            nc.tensor.matmul(
                ps[:, 0:512], ones1[:].bitcast(MM_DT), bp_t[0:1, 0:512],
                start=False, stop=True,
            )
            nc.tensor.matmul(
                ps[:, 512:768], ones1[:].bitcast(MM_DT), bp_t[0:1, 512:768],
                start=False, stop=True,
            )
            # x1 = x + attn_out + bp, overwriting the resident x chunk
            nc.vector.tensor_add(out=x1[tci][:], in0=ps[:], in1=x1[tci][:])
            # LN2 stats for this chunk (sqrt batched later: Exp stays the
            # loaded ACT table during attention)
            stats = sta.tile([P, 2, 6], F32, tag="stats")
            for s in range(2):
                nc.vector.bn_stats(
                    out=stats[:, s, :],
                    in_=x1[tci][:, s * 384 : (s + 1) * 384],
                )
            nc.vector.bn_aggr(out=mvs2[:, tci, :], in_=stats[:])

        eS1 = {}
        for h in range(H):
            emit_av(0, h, eS0[h])
            if h % 2 == 1:
                hh1 = (h - 1) // 2
                eS1[hh1] = emit_scores(1, hh1)
                # after both heads' j0 AV: bf16 redo of q-rows [0:128)
                emit_corr_pair(hh1)
        for h in range(6):
            eS1[h + 6] = emit_scores(1, h + 6)
            emit_av(1, h, eS1[h])
            if h >= 1 and h <= 4:
                emit_attnproj(h - 1)
        for h in range(6, H):
            emit_av(1, h, eS1[h])
        # Sqrt table prefetch ordered after the last exp
        nc.scalar.activation(
            out=scratch[:], in_=last_eS[0][:, 1, 511:512],
            func=mybir.ActivationFunctionType.Sqrt, scale=1.0,
        )

        att2_ctx.close()
        att_ctx.close()

        cm_ctx.close()
        w1_ctx.close()
        zt1_ctx.close()
        qkv_ctx.close()

        # ---- attn_proj t4-7 overlapped with LN2 first half ----------------
        mlp_ps_ctx = ExitStack()
        ps_fc = mlp_ps_ctx.enter_context(
            tc.tile_pool(name="ps_fc", bufs=2, space="PSUM")
        )
        attB_ctx = ExitStack()
        ps_apB = attB_ctx.enter_context(
            tc.tile_pool(name="ps_apB", bufs=2, space="PSUM")
        )
        ln2z = attB_ctx.enter_context(tc.tile_pool(name="ln2z", bufs=3))
        ln2pt = attB_ctx.enter_context(
            tc.tile_pool(name="ln2pt", bufs=2, space="PSUM")
        )
        ap_pool[0] = ps_apB

        def emit_ln2(tci):
            z = ln2z.tile([P, C], BF16, tag="z")
            for hh in range(2):
                nc.vector.tensor_scalar(
                    out=z[:, hh * 384 : (hh + 1) * 384],
                    in0=x1[tci][:, hh * 384 : (hh + 1) * 384],
                    scalar1=mvs2[:, tci, 0:1],
                    scalar2=rstds2[:, tci : tci + 1],
                    op0=mybir.AluOpType.subtract,
                    op1=mybir.AluOpType.mult,
                )
            # transposes batched into one PSUM tile; single strided copies
            # peel z2 into hi (fp8) + lo (residual, fp8) halves
            pt6 = ln2pt.tile([P, NCC, P], BF16, tag="pt")
            for cc in range(NCC):
                nc.tensor.transpose(
                    pt6[:, cc, :], z[:, cc * P : (cc + 1) * P], ident[:]
                )
            tsl = slice(tci * P, (tci + 1) * P)
            nc.scalar.copy(out=zt2h[:, :, tsl], in_=pt6[:])
            nc.vector.tensor_tensor(
                out=zt2l[:, :, tsl], in0=pt6[:], in1=zt2h[:, :, tsl],
                op=mybir.AluOpType.subtract,
            )

        # first-half rstd (all exps are done: single Sqrt table load)
        nc.scalar.activation(
            out=rstds2[:, 0:4],
            in_=mvs2[:, 0:4, 1],
            func=mybir.ActivationFunctionType.Sqrt,
            bias=eps_t[:],
            scale=1.0,
        )
        nc.vector.reciprocal(out=rstds2[:, 0:4], in_=rstds2[:, 0:4])
        for tci in range(4, NT):
            emit_ln2(tci - 4)
            emit_attnproj(tci)
        nc.scalar.activation(
            out=rstds2[:, 4:8],
            in_=mvs2[:, 4:8, 1],
            func=mybir.ActivationFunctionType.Sqrt,
            bias=eps_t[:],
            scale=1.0,
        )
        nc.vector.reciprocal(out=rstds2[:, 4:8], in_=rstds2[:, 4:8])
        for tci in range(4, NT):
            emit_ln2(tci)

        attB_ctx.close()

        # ---- stage 4+5: MLP ------------------------------------------------
        wm_ctx = ExitStack()
        wmp = wm_ctx.enter_context(tc.tile_pool(name="wmp", bufs=4))

        def load_wm_half(half):
            wmht = wmp.tile([P, NFH, C], FP8, tag="wms", name=f"wmh_{half}")
            wmlt = wmp.tile([P, NFH, C], FP8, tag="wms", name=f"wml_{half}")
            for kc in range(NFH):
                nc.sync.dma_start(wmht[:, kc, :], wmhr[:, half * NFH + kc, :])
                nc.sync.dma_start(wmlt[:, kc, :], wmlr[:, half * NFH + kc, :])
            return wmht, wmlt

        wmh0 = load_wm_half(0)
        # prefetch the Gelu table while the FC matmuls accumulate
        nc.scalar.activation(
            out=scratch[:], in_=rstds2[:, 4:5],
            func=mybir.ActivationFunctionType.Gelu_apprx_tanh, scale=1.0,
        )
        with (
            tc.tile_pool(name="mlpc", bufs=1) as mlpc,
            tc.tile_pool(name="gtp", bufs=1) as gtp,
            tc.tile_pool(name="gq", bufs=3) as gqp,
            tc.tile_pool(name="ps_mlp", bufs=3, space="PSUM") as ps_mlp,
        ):
            bm_b = mlpc.tile([P, C], F32)
            nc.gpsimd.dma_start(bm_b[:], bcast_ap(bm_h.ap()))

            for half in range(2):
                w2ht, w2lt = w2h0 if half == 0 else load_w2_half(1)
                wmht, wmlt = wmh0 if half == 0 else load_wm_half(1)
                gTh = gtp.tile([P, NFH, T], FP8, tag="gTh", name=f"gTh_{half}")
                gTl = gtp.tile([P, NFH, T], FP8, tag="gTl", name=f"gTl_{half}")
                for j in range(2):
                    for mf in range(NFH):
                        fc_glob = half * NFH + mf
                        ms = slice(mf * P, (mf + 1) * P)
                        js = slice(j * 512, (j + 1) * 512)
                        ps = ps_fc.tile([P, 512], F32, tag="psfc")
                        n = 0
                        for wt, zt in (
                            (w2ht, zt2h), (w2ht, zt2l), (w2lt, zt2h),
                        ):
                            for kp in range(NCC // 2):
                                nc.tensor.matmul(
                                    ps[:],
                                    wt[:, 2 * kp : 2 * kp + 2, ms],
                                    zt[:, 2 * kp : 2 * kp + 2, js],
                                    start=(n == 0),
                                    stop=(n == 8),
                                    perf_mode=DR,
                                )
                                n += 1
                        # gelu twice on ACT (fp8-hi + bf16 exact); DVE peels
                        # the residual into gTl.  scale 1/16 undoes the w2
                        # host pre-scale.
                        gq = gqp.tile([P, 512], BF16, tag="gq")
                        nc.scalar.activation(
                            out=gTh[:, mf, js],
                            in_=ps[:],
                            func=mybir.ActivationFunctionType.Gelu_apprx_tanh,
                            bias=b2c[:, fc_glob : fc_glob + 1],
                            scale=1.0 / WSCALE,
                        )
                        nc.scalar.activation(
                            out=gq[:],
                            in_=ps[:],
                            func=mybir.ActivationFunctionType.Gelu_apprx_tanh,
                            bias=b2c[:, fc_glob : fc_glob + 1],
                            scale=1.0 / WSCALE,
                        )
                        nc.vector.tensor_tensor(
                            out=gTl[:, mf, js], in0=gq[:], in1=gTh[:, mf, js],
                            op=mybir.AluOpType.subtract,
                        )
                for grp in ((0,), (1,), (2,), (3,), (4,), (5,), (6,), (7,)):
                    pss = {}
                    for tci in grp:
                        psm = ps_mlp.tile(
                            [P, C], F32, tag="psmlp", name=f"psm_{half}_{tci}"
                        )
                        pss[tci] = psm
                    for tci in grp:
                        tsl = slice(tci * P, (tci + 1) * P)
                        m = 0
                        for gt, wt in (
                            (gTh, wmht), (gTl, wmht), (gTh, wmlt),
                        ):
                            for kp in range(NFH // 2):
                                nc.tensor.matmul(
                                    pss[tci][:, 0:512],
                                    gt[:, 2 * kp : 2 * kp + 2, tsl],
                                    wt[:, 2 * kp : 2 * kp + 2, 0:512],
                                    start=(m == 0),
                                    stop=(m == 17),
                                    perf_mode=DR,
                                )
                                nc.tensor.matmul(
                                    pss[tci][:, 512:768],
                                    gt[:, 2 * kp : 2 * kp + 2, tsl],
                                    wt[:, 2 * kp : 2 * kp + 2, 512:768],
                                    start=(m == 0),
                                    stop=(m == 17),
                                    perf_mode=DR,
                                )
                                m += 1
                    for tci in grp:
                        # fused (psum * 1/16) + x1 on DVE undoes the wm x16
                        # pre-scale while draining PSUM
                        nc.vector.scalar_tensor_tensor(
                            out=x1[tci][:], in0=pss[tci][:],
                            scalar=1.0 / WSCALE, in1=x1[tci][:],
                            op0=mybir.AluOpType.mult,
                            op1=mybir.AluOpType.add,
                        )
                        if half == 0:
                            # bm on DVE: the MLP window is PE-bound, DVE idle
                            nc.vector.tensor_add(
                                out=x1[tci][:], in0=x1[tci][:], in1=bm_b[:]
                            )
                        else:
                            nc.sync.dma_start(yr[:, tci, :], x1[tci][:])

        wm_ctx.close()
        mlp_ps_ctx.close()
        wpp_ctx.close()
        ctp_ctx.close()
        mlpw_ctx.close()
        xp_ctx.close()

    nc.compile()
    return nc


# ---------------------------------------------------------------------------
# host wrapper
# ---------------------------------------------------------------------------

_module_cache: dict = {}
_module_lock = threading.Lock()


def _get_module(dbg: bool = False) -> bass.Bass:
    with _module_lock:
        if dbg not in _module_cache:
            _module_cache[dbg] = build_module(dbg)
        return _module_cache[dbg]


def _fold_inputs(
    x, ln1_scale, ln1_bias, w_qkv, b_qkv, w_attn_proj, b_attn_proj,
    ln2_scale, ln2_bias, w_fc, b_fc, w_mlp_proj, b_mlp_proj,
):
    import ml_dtypes

    f32 = np.float32
    bf16 = ml_dtypes.bfloat16
    fp8 = ml_dtypes.float8_e4m3
    w1 = (ln1_scale[:, None].astype(np.float64) * w_qkv.astype(np.float64)).astype(f32)
    b1 = (b_qkv.astype(np.float64) + ln1_bias.astype(np.float64) @ w_qkv.astype(np.float64)).astype(f32)
    w2 = (ln2_scale[:, None].astype(np.float64) * w_fc.astype(np.float64)).astype(f32)
    b2 = (b_fc.astype(np.float64) + ln2_bias.astype(np.float64) @ w_fc.astype(np.float64)).astype(f32)
    def hilo(w):
        # x16 pre-scale keeps the residual (lo) part of these ~N(0, 1/sqrt
        # (fan_in)) weights clear of the fp8e4 denormal floor
        ws = w.astype(np.float64) * WSCALE
        hi = ws.astype(f32).astype(fp8)
        lo = (ws - hi.astype(np.float64)).astype(f32).astype(fp8)
        return np.ascontiguousarray(hi), np.ascontiguousarray(lo)

    w2h, w2l = hilo(w2)
    wmh, wml = hilo(w_mlp_proj.astype(f32))
    w1h, w1l = hilo(w1)
    shared = {
        "w1h": w1h,
        "w1l": w1l,
        "b1": np.ascontiguousarray(b1 * np.float32(WSCALE)),
        "b1v": np.ascontiguousarray(
            (b1[2 * C : 3 * C] * np.float32(WSCALE)).astype(bf16)
        ),
        "wpb": np.ascontiguousarray(
            (w_attn_proj.astype(f32) / np.float32(WSCALE)).astype(bf16)
        ),
        "bp": np.ascontiguousarray(b_attn_proj.astype(f32)),
        "w2h": w2h,
        "w2l": w2l,
        "b2": np.ascontiguousarray(b2),
        "wmh": wmh,
        "wml": wml,
        "bm": np.ascontiguousarray(b_mlp_proj.astype(f32)),
    }
    return [
        {"x": np.ascontiguousarray(x[b].astype(f32).astype(bf16)), **shared} for b in range(B)
    ]


def run(inputs: dict, dbg: bool = False, **spmd_kwargs):
    """Run on 8 cores; returns BassKernelResults."""
    args = {k: np.asarray(v) for k, v in inputs.items()}
    in_maps = _fold_inputs(
        args["x"], args["ln1_scale"], args["ln1_bias"], args["w_qkv"],
        args["b_qkv"], args["w_attn_proj"], args["b_attn_proj"],
        args["ln2_scale"], args["ln2_bias"], args["w_fc"], args["b_fc"],
        args["w_mlp_proj"], args["b_mlp_proj"],
    )
    nc = _get_module(dbg)
    res = run_bass_kernel_spmd(nc, in_maps, core_ids=list(range(B)), **spmd_kwargs)
    return res


def kernel(**inputs) -> np.ndarray:
    res = run(inputs)
    return np.stack([res.results[b]["y"] for b in range(B)], axis=0).astype(
        np.float32
    )


if __name__ == "__main__":
    build_module(dbg=False)
    print("module built OK")


# revision 67
# speedup vs baseline: 1.1265x; 1.0015x over previous
"""Transformer block (pre-LN attention + MLP) for B=8, T=1024, C=768, H=12.

Sharding: pure data-parallel — one batch element per NeuronCore, identical
SPMD program on cores 0-7, no collectives.

Per-core dataflow (activations kept on-chip end to end):
  x (resident, loaded once; x1 written in place) -> LN1 -> PE-transpose
    -> zt1 fp8 [C, T] -> QKV fp8 DoubleRow (w1 fp8 stationary): qT/kT bf16
    head-major, v fp8 natural (+ones col)
    -> attention: S^T = K Q^T (kT bf16 stationary x qT bf16 moving, causal
       block-skip), exp on ACT (bias -1.5, cancelled by normalization) ->
       per-pair eS fp8 tiles, staircase causal masks on DVE (widened on the
       second-of-pair tile so the pair's dead region is zeroed for free),
       AV fp8 DoubleRow (vnat fp8 stationary x eS fp8 pairs moving) -> ctx^T
       + row-sums, normalize via DVE reciprocal + gpsimd bcast -> ctxT fp8
    -> attn_proj fp8 DoubleRow (ctxT stationary x wp fp8 moving) + residual
    -> LN2 -> zt2 f32r -> FC (w2 f32r x zt2 f32r, pure; gelu fused on ACT)
    -> gT f32r -> MLP proj (gT stationary x wm f32r moving, pure) -> y

fp8e4 DoubleRow (2 k-subtiles per matmul at 0.5 cycles/row) quarters the
PE time of the K-deep attention GEMMs; scores stay bf16 (K=64 per head
cannot pair, and bf16 costs the same as fp8 there while keeping q/k exact).
"""

import threading
from contextlib import ExitStack

import numpy as np

import concourse.bass as bass
from concourse import bacc
import concourse.mybir as mybir
import concourse.tile as tile
from concourse.bass_utils import run_bass_kernel_spmd
from concourse.masks import make_identity


# ---------------------------------------------------------------------------

B, T, C, H, HD, F, P = 8, 1024, 768, 12, 64, 3072, 128
NT = T // P        # 8  t-chunks
NCC = C // P       # 6  c-chunks
NFH = F // 2 // P  # 12 f-chunks per half
EPS = 1e-5
# weights are pre-scaled x16 on the host so their hi/lo fp8e4 splits stay
# clear of the e4m3 denormal floor; q,k each carry x16 so the score scale
# absorbs 1/256, and wp carries the 1/16 for the x16 in v/ctx.
WSCALE = 16.0
ATTN_SCALE = 1.0 / (8.0 * WSCALE * WSCALE)
EXP_BIAS = -2.75   # exp(S*scale - 2.75): keeps eS under fp8e4 max (240)
                   # even for ~8-sigma scores; a per-row-constant shift
                   # cancels in the softmax ratio.

F32 = mybir.dt.float32
MM_DT = mybir.dt.float32r
BF16 = mybir.dt.bfloat16
FP8 = mybir.dt.float8e4
DR = mybir.MatmulPerfMode.DoubleRow


def build_module(dbg: bool = False) -> bass.Bass:
    nc = bacc.Bacc()

    x_h = nc.dram_tensor("x", [T, C], BF16, kind="ExternalInput")
    w1h_h = nc.dram_tensor("w1h", [C, 3 * C], FP8, kind="ExternalInput")
    w1l_h = nc.dram_tensor("w1l", [C, 3 * C], FP8, kind="ExternalInput")
    b1_h = nc.dram_tensor("b1", [3 * C], F32, kind="ExternalInput")
    b1v_h = nc.dram_tensor("b1v", [C], BF16, kind="ExternalInput")
    wpb_h = nc.dram_tensor("wpb", [C, C], BF16, kind="ExternalInput")
    bp_h = nc.dram_tensor("bp", [C], F32, kind="ExternalInput")
    w2h_h = nc.dram_tensor("w2h", [C, F], FP8, kind="ExternalInput")
    w2l_h = nc.dram_tensor("w2l", [C, F], FP8, kind="ExternalInput")
    b2_h = nc.dram_tensor("b2", [F], F32, kind="ExternalInput")
    wmh_h = nc.dram_tensor("wmh", [F, C], FP8, kind="ExternalInput")
    wml_h = nc.dram_tensor("wml", [F, C], FP8, kind="ExternalInput")
    bm_h = nc.dram_tensor("bm", [C], F32, kind="ExternalInput")
    y_h = nc.dram_tensor("y", [T, C], BF16, kind="ExternalOutput")

    def bcast_ap(ap1d, n_part=P):
        return bass.AP(
            tensor=ap1d.tensor, offset=ap1d.offset, ap=[[0, n_part], *ap1d.ap]
        )

    xr = x_h.ap().rearrange("(tc p) c -> p tc c", p=P)
    yr = y_h.ap().rearrange("(tc p) c -> p tc c", p=P)
    w1hr = w1h_h.ap().rearrange("(kc p) d -> p kc d", p=P)
    w1lr = w1l_h.ap().rearrange("(kc p) d -> p kc d", p=P)
    w2hr = w2h_h.ap().rearrange("(kc p) f -> p kc f", p=P)
    w2lr = w2l_h.ap().rearrange("(kc p) f -> p kc f", p=P)
    wmhr = wmh_h.ap().rearrange("(fc p) c -> p fc c", p=P)
    wmlr = wml_h.ap().rearrange("(fc p) c -> p fc c", p=P)

    with tile.TileContext(nc) as tc, ExitStack() as top:
        consts = top.enter_context(tc.tile_pool(name="consts", bufs=1))

        ident = consts.tile([P, P], BF16)
        make_identity(nc, ident[:])
        eps_t = consts.tile([P, 1], F32)
        nc.vector.memset(eps_t[:], EPS)
        negone = consts.tile([P, 1], F32)
        nc.vector.memset(negone[:], EXP_BIAS)
        ones_col = consts.tile([P, 1], F32)
        nc.vector.memset(ones_col[:], 1.0)
        # K=1 broadcast / zeroing helpers for the PE
        ones64 = consts.tile([1, HD], BF16)
        nc.vector.memset(ones64[:], 1.0)
        # dead-region fill: ones-col x (-1e6) row makes exp underflow to 0
        zrow = consts.tile([1, 512], BF16)
        nc.vector.memset(zrow[:], -1.0e6)
        zcol = consts.tile([1, P], BF16)
        nc.vector.memset(zcol[:], 1.0)
        scratch = consts.tile([P, 1], F32)
        # prefetch the Sqrt act-table while the first x chunk is in flight
        nc.scalar.activation(
            out=scratch[:], in_=eps_t[:],
            func=mybir.ActivationFunctionType.Sqrt, scale=1.0,
        )
        b1qk = consts.tile([P, 2 * C // P], F32)
        b2c = consts.tile([P, F // P], F32)

        # ---- long-lived pools (stack allocator: open order = close order
        # reversed) --------------------------------------------------------
        xp_ctx = ExitStack()
        xp = xp_ctx.enter_context(tc.tile_pool(name="xp", bufs=1))
        # x chunks, overwritten in place by x1 = x + attn_out at stage 3
        x1 = [
            xp.tile([P, C], BF16, tag=f"x_{i}", name=f"xc_{i}")
            for i in range(NT)
        ]

        mlpw_ctx = ExitStack()
        w2p = mlpw_ctx.enter_context(tc.tile_pool(name="w2p", bufs=4))
        lnm = mlpw_ctx.enter_context(tc.tile_pool(name="lnm", bufs=1))
        sta = mlpw_ctx.enter_context(tc.tile_pool(name="sta", bufs=3))
        mvs2 = lnm.tile([P, NT, 2], F32)
        rstds2 = lnm.tile([P, NT], F32)
        zt2p = mlpw_ctx.enter_context(tc.tile_pool(name="zt2p", bufs=1))
        zt2h = zt2p.tile([P, NCC, T], FP8)
        zt2l = zt2p.tile([P, NCC, T], FP8)

        ctp_ctx = ExitStack()
        ctp = ctp_ctx.enter_context(tc.tile_pool(name="ctp", bufs=1))
        ctxT = ctp.tile([P, NCC, T], BF16)

        wpp_ctx = ExitStack()
        wpp = wpp_ctx.enter_context(tc.tile_pool(name="wpp", bufs=1))
        wp_b = wpp.tile([P, NCC, C], BF16)
        bp_t = wpp.tile([1, C], MM_DT)
        ones1 = wpp.tile([1, P], F32)

        qkv_ctx = ExitStack()
        qkp = qkv_ctx.enter_context(tc.tile_pool(name="qkp", bufs=1))
        vp = qkv_ctx.enter_context(tc.tile_pool(name="vp", bufs=1))
        qT_t = qkp.tile([P, NCC, T], BF16)
        kT_t = qkp.tile([P, NCC, T], BF16)
        # v in natural layout, heads at stride HD+1 (ones col for row-sums);
        # t-chunk stride padded to a 16B multiple (DoubleRow ldweights
        # requires pair-dim stride % 16 == 0)
        VSTR = 784  # >= H * (HD + 1) = 780, multiple of 16
        vnat = vp.tile([P, NT, VSTR], FP8)
        # bf16 v for t-chunk 0 (feeds the bf16 early-row attention path)
        v0c = vp.tile([P, H * (HD + 1)], BF16)
        b1v_b = vp.tile([P, C], BF16)

        zt1_ctx = ExitStack()
        zt1p = zt1_ctx.enter_context(tc.tile_pool(name="zt1p", bufs=1))
        zt1h = zt1p.tile([P, NCC, T], FP8)
        zt1l = zt1p.tile([P, NCC, T], FP8)

        w1_ctx = ExitStack()
        w1p = w1_ctx.enter_context(tc.tile_pool(name="w1p", bufs=1))
        w1qh = w1p.tile([P, NCC, C], FP8)
        w1ql = w1p.tile([P, NCC, C], FP8)
        w1kh = w1p.tile([P, NCC, C], FP8)
        w1kl = w1p.tile([P, NCC, C], FP8)
        w1vh = w1p.tile([P, NCC, C], FP8)
        w1vl = w1p.tile([P, NCC, C], FP8)

        # ---- startup DMA interleave: x chunks + w1 q/k passes -------------
        # tiny bias/const loads first so nothing downstream queues behind a
        # buffer-rotation stall
        nc.sync.dma_start(
            b1qk[:], b1_h.ap()[0 : 2 * C].rearrange("(dc p) -> p dc", p=P)
        )
        nc.sync.dma_start(b2c[:], b2_h.ap().rearrange("(fc p) -> p fc", p=P))
        nc.sync.dma_start(bp_t[0:1, :], bp_h.ap().bitcast(MM_DT))
        nc.vector.memset(ones1[:], 1.0)
        nc.sync.dma_start(x1[0][:], xr[:, 0, :])
        nc.sync.dma_start(x1[1][:], xr[:, 1, :])
        for kc in range(NCC):
            if kc + 2 < NT:
                nc.sync.dma_start(x1[kc + 2][:], xr[:, kc + 2, :])
            nc.sync.dma_start(w1qh[:, kc, :], w1hr[:, kc, 0:C])
            nc.sync.dma_start(w1ql[:, kc, :], w1lr[:, kc, 0:C])
        for kc in range(NCC):
            nc.sync.dma_start(w1kh[:, kc, :], w1hr[:, kc, C : 2 * C])
            nc.sync.dma_start(w1kl[:, kc, :], w1lr[:, kc, C : 2 * C])
        nc.gpsimd.dma_start(b1v_b[:], bcast_ap(b1v_h.ap()))

        def load_w1v():
            for kc in range(NCC):
                nc.sync.dma_start(w1vh[:, kc, :], w1hr[:, kc, 2 * C : 3 * C])
                nc.sync.dma_start(w1vl[:, kc, :], w1lr[:, kc, 2 * C : 3 * C])

        # ---- LN + transpose helper (batched copies, hi/lo peel) -----------
        last_rstd = [None]

        def layernorm_transpose(get_src, zth, ztl, zpool, spool, pst):
            for tci in range(NT):
                xt = get_src(tci)
                stats = spool.tile([P, 2, 6], F32, tag="stats")
                for s in range(2):
                    nc.vector.bn_stats(
                        out=stats[:, s, :], in_=xt[:, s * 384 : (s + 1) * 384]
                    )
                mv = spool.tile([P, 2], F32, tag="mv")
                nc.vector.bn_aggr(out=mv[:], in_=stats[:])
                rstd = spool.tile([P, 1], F32, tag="rstd")
                nc.scalar.activation(
                    out=rstd[:],
                    in_=mv[:, 1:2],
                    func=mybir.ActivationFunctionType.Sqrt,
                    bias=eps_t[:],
                    scale=1.0,
                )
                nc.vector.reciprocal(out=rstd[:], in_=rstd[:])
                last_rstd[0] = rstd
                z = zpool.tile([P, C], BF16, tag="z")
                for hh in range(2):
                    nc.vector.tensor_scalar(
                        out=z[:, hh * 384 : (hh + 1) * 384],
                        in0=xt[:, hh * 384 : (hh + 1) * 384],
                        scalar1=mv[:, 0:1],
                        scalar2=rstd[:],
                        op0=mybir.AluOpType.subtract,
                        op1=mybir.AluOpType.mult,
                    )
                pt6 = pst.tile([P, NCC, P], BF16, tag="pt")
                for cc in range(NCC):
                    nc.tensor.transpose(
                        pt6[:, cc, :], z[:, cc * P : (cc + 1) * P], ident[:]
                    )
                tsl = slice(tci * P, (tci + 1) * P)
                nc.scalar.copy(out=zth[:, :, tsl], in_=pt6[:])
                if ztl is not None:
                    nc.vector.tensor_tensor(
                        out=ztl[:, :, tsl], in0=pt6[:], in1=zth[:, :, tsl],
                        op=mybir.AluOpType.subtract,
                    )

        # ---- stage 1a: LN1 (x already resident) ---------------------------
        with (
            tc.tile_pool(name="ln_z", bufs=3) as _zp,
            tc.tile_pool(name="ln_s", bufs=3) as _sp,
            tc.tile_pool(name="ln_pt", bufs=3, space="PSUM") as _pp,
        ):
            layernorm_transpose(lambda t: x1[t][:], zt1h, zt1l, _zp, _sp, _pp)


        def load_w2_half(half):
            w2ht = w2p.tile([P, NCC, F // 2], FP8, tag="w2s",
                            name=f"w2h_{half}")
            w2lt = w2p.tile([P, NCC, F // 2], FP8, tag="w2s",
                            name=f"w2l_{half}")
            fs = slice(half * (F // 2), (half + 1) * (F // 2))
            for kc in range(NCC):
                nc.sync.dma_start(w2ht[:, kc, :], w2hr[:, kc, fs])
                nc.sync.dma_start(w2lt[:, kc, :], w2lr[:, kc, fs])
            return w2ht, w2lt


        # v weights before the mlp prefetches: the sync DMA queue is FIFO, so
        # anything QKV needs must precede DMAs that can stall on buffer reuse
        load_w1v()

        # ---- attention pools that span the QKV merge ----------------------
        cm_ctx = ExitStack()
        att_ctx = ExitStack()
        esp = att_ctx.enter_context(tc.tile_pool(name="esp", bufs=26))
        rcp = att_ctx.enter_context(tc.tile_pool(name="rcp", bufs=2))
        bcp = att_ctx.enter_context(tc.tile_pool(name="bcp", bufs=2))
        cep = att_ctx.enter_context(tc.tile_pool(name="cep", bufs=3))
        ps_s = att_ctx.enter_context(
            tc.tile_pool(name="ps_s", bufs=2, space="PSUM")
        )

        last_eS = [None]

        def emit_scores(j, h):
            """Scores + exp for head h, q-half j. Returns the eS pair tiles."""
            row = (h % 2) * HD
            qT_h = qT_t[row : row + HD, h // 2, :]
            kT_h = kT_t[row : row + HD, h // 2, :]
            eS = []
            # full (non-diagonal) k-chunks, exp batched in pairs
            for half in range(2 * j):
                psS = ps_s.tile([P, 2, 512], F32, tag="psS")
                eSt = esp.tile([P, 2, 512], FP8, tag="eS")
                for mi in range(2):
                    m = half * 2 + mi
                    nc.tensor.matmul(
                        psS[:, mi, :],
                        (kT_h[:, m * P : (m + 1) * P]),
                        (qT_h[:, j * 512 : (j + 1) * 512]),
                        start=True,
                        stop=True,
                    )
                nc.scalar.activation(
                    out=eSt[:],
                    in_=psS[:],
                    func=mybir.ActivationFunctionType.Exp,
                    bias=negone[:],
                    scale=ATTN_SCALE,
                )
                eS.append(eSt)
            # diagonal-crossing k-chunk pairs: the pair is exp'd in ONE op
            # over [rp:512]; the second tile's dead psum block [rp:r) is
            # zeroed on the PE (K=1 x zeros) so exp sees defined data, and
            # its widened staircase select zeroes the eS dead region.
            for di in range(2):
                rp = max(P if j == 0 else 0,
                         (4 * j + di * 2) * P - 512 * j)
                psS = ps_s.tile([P, 2, 512], F32, tag="psS")
                eSt = esp.tile([P, 2, 512], FP8, tag="eS")
                for mi in range(2):
                    m = 4 * j + di * 2 + mi
                    r = max(rp, m * P - 512 * j)
                    if r > rp:
                        nc.tensor.matmul(
                            psS[:, mi, rp:r], zcol[:], zrow[:, 0 : r - rp],
                            start=True, stop=True,
                        )
                    nc.tensor.matmul(
                        psS[:, mi, r:512],
                        (kT_h[:, m * P : (m + 1) * P]),
                        (qT_h[:, j * 512 + r : (j + 1) * 512]),
                        start=True,
                        stop=True,
                    )
                nc.scalar.activation(
                    out=eSt[:, :, rp:512],
                    in_=psS[:, :, rp:512],
                    func=mybir.ActivationFunctionType.Exp,
                    bias=negone[:],
                    scale=ATTN_SCALE,
                )
                for mi in range(2):
                    m = 4 * j + di * 2 + mi
                    r = m * P - 512 * j
                    if r < rp:
                        continue  # staircase block lives in the skipped cols
                    nc.gpsimd.affine_select(
                        out=eSt[:, mi, r : r + P],
                        in_=eSt[:, mi, r : r + P],
                        compare_op=mybir.AluOpType.is_ge,
                        fill=0.0,
                        base=0,
                        pattern=[[1, P]],
                        channel_multiplier=-1,
                    )
                eS.append(eSt)
            last_eS[0] = eS[-1]
            return eS

        def emit_av(j, h, eS):
            row = (h % 2) * HD
            np_ = 2 * (j + 1)
            q0 = P if j == 0 else 0  # q-block 0 comes from the bf16 redo
            psC = ps_c.tile([HD + 1, 512], F32, tag="psC")
            for pi in range(np_):
                r = max(q0, 2 * pi * P - 512 * j)
                nc.tensor.matmul(
                    psC[:, r:512],
                    (vnat[:, 2 * pi : 2 * pi + 2,
                          h * (HD + 1) : h * (HD + 1) + HD + 1]),
                    (eS[pi][:, :, r:512]),
                    start=(pi == 0),
                    stop=(pi == np_ - 1),
                    perf_mode=DR,
                )
            recip = rcp.tile([1, 512], BF16, tag="recip")
            with nc.allow_low_precision(reason="softmax scale in bf16"):
                nc.vector.reciprocal(
                    out=recip[:, q0:512], in_=psC[HD : HD + 1, q0:512]
                )
            bc = bcp.tile([HD, 512], BF16, tag="bc")
            nc.gpsimd.partition_broadcast(bc[:, q0:512], recip[:, q0:512])
            nc.vector.tensor_mul(
                out=ctxT[row : row + HD, h // 2,
                         j * 512 + q0 : (j + 1) * 512],
                in0=psC[0:HD, q0:512],
                in1=bc[:, q0:512],
            )

        # ---- stage 1b: QKV merged with j=0 scores/exp ----------------------
        with (
            tc.tile_pool(name="ps_qk", bufs=2, space="PSUM") as ps_qk,
            tc.tile_pool(name="ps_v", bufs=1, space="PSUM") as ps_v,
        ):
            def emit_qk(qk, dcl):
                w1s_h = w1qh if qk == 0 else w1kh
                w1s_l = w1ql if qk == 0 else w1kl
                dc = qk * NCC + dcl
                dst_t = qT_t if qk == 0 else kT_t
                dsl = slice(dcl * P, (dcl + 1) * P)
                for j in range(2):
                    jsl = slice(j * 512, (j + 1) * 512)
                    ps = ps_qk.tile([P, 512], F32, tag="psqk")
                    m = 0
                    for ws, zs in (
                        (w1s_h, zt1h), (w1s_h, zt1l), (w1s_l, zt1h),
                    ):
                        for kp in range(NCC // 2):
                            nc.tensor.matmul(
                                ps[:],
                                (ws[:, 2 * kp : 2 * kp + 2, dsl]),
                                (zs[:, 2 * kp : 2 * kp + 2, jsl]),
                                start=(m == 0),
                                stop=(m == 8),
                                perf_mode=DR,
                            )
                            m += 1
                    nc.vector.tensor_scalar_add(
                        out=dst_t[:, dcl, jsl],
                        in0=ps[:],
                        scalar1=b1qk[:, dc : dc + 1],
                    )

            def emit_v_psum(tci, combos):
                psv5 = ps_v.tile([P, 512], F32, tag="psv5")
                psv2 = ps_v.tile([P, 256], F32, tag="psv2")
                n = len(combos) * (NCC // 2)
                m = 0
                for zs, ws in combos:
                    for kp in range(NCC // 2):
                        zsl = zs[:, 2 * kp : 2 * kp + 2,
                                 tci * P : (tci + 1) * P]
                        nc.tensor.matmul(
                            psv5[:], zsl,
                            (ws[:, 2 * kp : 2 * kp + 2, 0:512]),
                            start=(m == 0), stop=(m == n - 1), perf_mode=DR,
                        )
                        nc.tensor.matmul(
                            psv2[:], zsl,
                            (ws[:, 2 * kp : 2 * kp + 2, 512:768]),
                            start=(m == 0), stop=(m == n - 1), perf_mode=DR,
                        )
                        m += 1
                return psv5, psv2

            def write_v(vh, psv5, psv2):
                nc.vector.tensor_add(
                    out=vh[:, 0:8, 0:HD],
                    in0=psv5[:].rearrange("p (h d) -> p h d", h=8),
                    in1=b1v_b[:, 0:512].rearrange("p (h d) -> p h d", h=8),
                )
                nc.vector.tensor_add(
                    out=vh[:, 8:12, 0:HD],
                    in0=psv2[:].rearrange("p (h d) -> p h d", h=4),
                    in1=b1v_b[:, 512:768].rearrange("p (h d) -> p h d", h=4),
                )
                nc.vector.tensor_copy(
                    out=vh[:, :, HD : HD + 1].rearrange(
                        "p h one -> p (h one)"
                    ),
                    in_=ones_col[:].to_broadcast((P, H)),
                )

            def emit_v(tci):
                psv5, psv2 = emit_v_psum(tci, ((zt1h, w1vh),))
                vh = vnat[:, tci, 0 : H * (HD + 1)].rearrange(
                    "p (h e) -> p h e", h=H
                )
                write_v(vh, psv5, psv2)

            def emit_v0c():
                # bf16-grade v for t-chunk 0 (hi/lo corrected)
                psv5, psv2 = emit_v_psum(
                    0, ((zt1h, w1vh), (zt1l, w1vh), (zt1h, w1vl))
                )
                vh = v0c[:].rearrange("p (h e) -> p h e", h=H)
                write_v(vh, psv5, psv2)

            emit_qk(0, 0)
            emit_qk(1, 0)
            for tci in range(4):
                emit_v(tci)
            # Exp table prefetch (input dep on LN1 t7 rstd orders it after
            # the last LN1 sqrt)
            nc.scalar.activation(
                out=scratch[:], in_=last_rstd[0][:],
                func=mybir.ActivationFunctionType.Exp, scale=1.0,
            )
            eS0 = {}
            eS0[0] = emit_scores(0, 0)
            eS0[1] = emit_scores(0, 1)
            for dcl in range(1, NCC):
                emit_qk(0, dcl)
                emit_qk(1, dcl)
                if dcl <= 4:
                    emit_v(3 + dcl)
                eS0[2 * dcl] = emit_scores(0, 2 * dcl)
                eS0[2 * dcl + 1] = emit_scores(0, 2 * dcl + 1)
            emit_v0c()

        # wp + h0 FC weights stream during attention (wm streams during FC)
        for kc in range(NCC):
            nc.sync.dma_start(
                wp_b[:, kc, :],
                wpb_h.ap().rearrange("(kc p) c -> p kc c", p=P)[:, kc, :],
            )
        w2h0 = load_w2_half(0)

        # ---- stage 2: AV(j0), j=1 heads, attn_proj, LN2 stats --------------
        att2_ctx = ExitStack()
        ps_c = att2_ctx.enter_context(
            tc.tile_pool(name="ps_c", bufs=2, space="PSUM")
        )
        ps_ap = att2_ctx.enter_context(
            tc.tile_pool(name="ps_ap", bufs=1, space="PSUM")
        )
        ap_pool = [ps_ap]

        def emit_corr_pair(hp):
            """bf16 attention for q-rows [0:128), head pair (2hp, 2hp+1)."""
            psS = ps_s.tile([P, 2, 512], F32, tag="psS")
            eS0c = cep.tile([P, 2, P], BF16, tag="eS0c")
            for mi in range(2):
                h = 2 * hp + mi
                row = (h % 2) * HD
                nc.tensor.matmul(
                    psS[:, mi, 0:P],
                    (kT_t[row : row + HD, hp, 0:P]),
                    (qT_t[row : row + HD, hp, 0:P]),
                    start=True,
                    stop=True,
                )
            nc.scalar.activation(
                out=eS0c[:],
                in_=psS[:, :, 0:P],
                func=mybir.ActivationFunctionType.Exp,
                bias=negone[:],
                scale=ATTN_SCALE,
            )
            for mi in range(2):
                h = 2 * hp + mi
                row = (h % 2) * HD
                nc.gpsimd.affine_select(
                    out=eS0c[:, mi, :],
                    in_=eS0c[:, mi, :],
                    compare_op=mybir.AluOpType.is_ge,
                    fill=0.0,
                    base=0,
                    pattern=[[1, P]],
                    channel_multiplier=-1,
                )
                psC0 = ps_c.tile([HD + 1, 512], F32, tag="psC")
                nc.tensor.matmul(
                    psC0[:, 0:P],
                    (v0c[:, h * (HD + 1) : (h + 1) * (HD + 1)]),
                    (eS0c[:, mi, :]),
                    start=True,
                    stop=True,
                )
                recip = rcp.tile([1, 512], BF16, tag="recip")
                with nc.allow_low_precision(reason="softmax scale in bf16"):
                    nc.vector.reciprocal(
                        out=recip[:, 0:P], in_=psC0[HD : HD + 1, 0:P]
                    )
                bc = bcp.tile([HD, 512], BF16, tag="bc")
                nc.gpsimd.partition_broadcast(bc[:, 0:P], recip[:, 0:P])
                # overwrites the fp8-grade main-path result for q in [0:128)
                nc.vector.tensor_mul(
                    out=ctxT[row : row + HD, hp, 0:P],
                    in0=psC0[0:HD, 0:P],
                    in1=bc[:, 0:P],
                )

        def emit_attnproj(tci):
            ps = ap_pool[0].tile([P, C], F32, tag="psap")
            for cc in range(NCC):
                nc.tensor.matmul(
                    ps[:, 0:512],
                    (ctxT[:, cc, tci * P : (tci + 1) * P]),
                    (wp_b[:, cc, 0:512]),
                    start=(cc == 0), stop=False,
                )
                nc.tensor.matmul(
                    ps[:, 512:768],
                    (ctxT[:, cc, tci * P : (tci + 1) * P]),
                    (wp_b[:, cc, 512:768]),
                    start=(cc == 0), stop=False,
                )
            # bp folded in as a K=1 rank-1 update (ones x bp)
            nc.tensor.matmul(
                ps[:, 0:512], ones1[:].bitcast(MM_DT), bp_t[0:1, 0:512],
                start=False, stop=True,
            )
            nc.tensor.matmul(
                ps[:, 512:768], ones1[:].bitcast(MM_DT), bp_t[0:1, 512:768],
                start=False, stop=True,
            )
            # x1 = x + attn_out + bp, overwriting the resident x chunk
            nc.vector.tensor_add(out=x1[tci][:], in0=ps[:], in1=x1[tci][:])
            # LN2 stats for this chunk (sqrt batched later: Exp stays the
            # loaded ACT table during attention)
            stats = sta.tile([P, 2, 6], F32, tag="stats")
            for s in range(2):
                nc.vector.bn_stats(
                    out=stats[:, s, :],
                    in_=x1[tci][:, s * 384 : (s + 1) * 384],
                )
            nc.vector.bn_aggr(out=mvs2[:, tci, :], in_=stats[:])

        eS1 = {}
        for h in range(H):
            emit_av(0, h, eS0[h])
            if h % 2 == 1:
                hh1 = (h - 1) // 2
                eS1[hh1] = emit_scores(1, hh1)
                # after both heads' j0 AV: bf16 redo of q-rows [0:128)
                emit_corr_pair(hh1)
        for h in range(6):
            eS1[h + 6] = emit_scores(1, h + 6)
            emit_av(1, h, eS1[h])
            if h <= 3:
                emit_attnproj(h)
        for h in range(6, H):
            emit_av(1, h, eS1[h])
        # Sqrt table prefetch ordered after the last exp
        nc.scalar.activation(
            out=scratch[:], in_=last_eS[0][:, 1, 511:512],
            func=mybir.ActivationFunctionType.Sqrt, scale=1.0,
        )

        att2_ctx.close()
        att_ctx.close()

        cm_ctx.close()
        w1_ctx.close()
        zt1_ctx.close()
        qkv_ctx.close()

        # ---- attn_proj t4-7 overlapped with LN2 first half ----------------
        mlp_ps_ctx = ExitStack()
        ps_fc = mlp_ps_ctx.enter_context(
            tc.tile_pool(name="ps_fc", bufs=2, space="PSUM")
        )
        attB_ctx = ExitStack()
        ps_apB = attB_ctx.enter_context(
            tc.tile_pool(name="ps_apB", bufs=2, space="PSUM")
        )
        ln2z = attB_ctx.enter_context(tc.tile_pool(name="ln2z", bufs=3))
        ln2pt = attB_ctx.enter_context(
            tc.tile_pool(name="ln2pt", bufs=2, space="PSUM")
        )
        ap_pool[0] = ps_apB

        def emit_ln2(tci):
            z = ln2z.tile([P, C], BF16, tag="z")
            for hh in range(2):
                nc.vector.tensor_scalar(
                    out=z[:, hh * 384 : (hh + 1) * 384],
                    in0=x1[tci][:, hh * 384 : (hh + 1) * 384],
                    scalar1=mvs2[:, tci, 0:1],
                    scalar2=rstds2[:, tci : tci + 1],
                    op0=mybir.AluOpType.subtract,
                    op1=mybir.AluOpType.mult,
                )
            # transposes batched into one PSUM tile; single strided copies
            # peel z2 into hi (fp8) + lo (residual, fp8) halves
            pt6 = ln2pt.tile([P, NCC, P], BF16, tag="pt")
            for cc in range(NCC):
                nc.tensor.transpose(
                    pt6[:, cc, :], z[:, cc * P : (cc + 1) * P], ident[:]
                )
            tsl = slice(tci * P, (tci + 1) * P)
            nc.scalar.copy(out=zt2h[:, :, tsl], in_=pt6[:])
            nc.vector.tensor_tensor(
                out=zt2l[:, :, tsl], in0=pt6[:], in1=zt2h[:, :, tsl],
                op=mybir.AluOpType.subtract,
            )

        # first-half rstd (all exps are done: single Sqrt table load)
        nc.scalar.activation(
            out=rstds2[:, 0:4],
            in_=mvs2[:, 0:4, 1],
            func=mybir.ActivationFunctionType.Sqrt,
            bias=eps_t[:],
            scale=1.0,
        )
        nc.vector.reciprocal(out=rstds2[:, 0:4], in_=rstds2[:, 0:4])
        for tci in range(4, NT):
            emit_ln2(tci - 4)
            emit_attnproj(tci)
        nc.scalar.activation(
            out=rstds2[:, 4:8],
            in_=mvs2[:, 4:8, 1],
            func=mybir.ActivationFunctionType.Sqrt,
            bias=eps_t[:],
            scale=1.0,
        )
        nc.vector.reciprocal(out=rstds2[:, 4:8], in_=rstds2[:, 4:8])
        for tci in range(4, NT):
            emit_ln2(tci)

        attB_ctx.close()

        # ---- stage 4+5: MLP ------------------------------------------------
        wm_ctx = ExitStack()
        wmp = wm_ctx.enter_context(tc.tile_pool(name="wmp", bufs=4))

        def load_wm_half(half):
            wmht = wmp.tile([P, NFH, C], FP8, tag="wms", name=f"wmh_{half}")
            wmlt = wmp.tile([P, NFH, C], FP8, tag="wms", name=f"wml_{half}")
            for kc in range(NFH):
                nc.sync.dma_start(wmht[:, kc, :], wmhr[:, half * NFH + kc, :])
                nc.sync.dma_start(wmlt[:, kc, :], wmlr[:, half * NFH + kc, :])
            return wmht, wmlt

        wmh0 = load_wm_half(0)
        # prefetch the Gelu table while the FC matmuls accumulate
        nc.scalar.activation(
            out=scratch[:], in_=rstds2[:, 4:5],
            func=mybir.ActivationFunctionType.Gelu_apprx_tanh, scale=1.0,
        )
        with (
            tc.tile_pool(name="mlpc", bufs=1) as mlpc,
            tc.tile_pool(name="gtp", bufs=1) as gtp,
            tc.tile_pool(name="gq", bufs=3) as gqp,
            tc.tile_pool(name="ps_mlp", bufs=3, space="PSUM") as ps_mlp,
        ):
            bm_b = mlpc.tile([P, C], F32)
            nc.gpsimd.dma_start(bm_b[:], bcast_ap(bm_h.ap()))

            for half in range(2):
                w2ht, w2lt = w2h0 if half == 0 else load_w2_half(1)
                wmht, wmlt = wmh0 if half == 0 else load_wm_half(1)
                gTh = gtp.tile([P, NFH, T], FP8, tag="gTh", name=f"gTh_{half}")
                gTl = gtp.tile([P, NFH, T], FP8, tag="gTl", name=f"gTl_{half}")
                for j in range(2):
                    for mf in range(NFH):
                        fc_glob = half * NFH + mf
                        ms = slice(mf * P, (mf + 1) * P)
                        js = slice(j * 512, (j + 1) * 512)
                        ps = ps_fc.tile([P, 512], F32, tag="psfc")
                        n = 0
                        for wt, zt in (
                            (w2ht, zt2h), (w2ht, zt2l), (w2lt, zt2h),
                        ):
                            for kp in range(NCC // 2):
                                nc.tensor.matmul(
                                    ps[:],
                                    wt[:, 2 * kp : 2 * kp + 2, ms],
                                    zt[:, 2 * kp : 2 * kp + 2, js],
                                    start=(n == 0),
                                    stop=(n == 8),
                                    perf_mode=DR,
                                )
                                n += 1
                        # gelu twice on ACT (fp8-hi + bf16 exact); DVE peels
                        # the residual into gTl.  scale 1/16 undoes the w2
                        # host pre-scale.
                        gq = gqp.tile([P, 512], BF16, tag="gq")
                        nc.scalar.activation(
                            out=gTh[:, mf, js],
                            in_=ps[:],
                            func=mybir.ActivationFunctionType.Gelu_apprx_tanh,
                            bias=b2c[:, fc_glob : fc_glob + 1],
                            scale=1.0 / WSCALE,
                        )
                        nc.scalar.activation(
                            out=gq[:],
                            in_=ps[:],
                            func=mybir.ActivationFunctionType.Gelu_apprx_tanh,
                            bias=b2c[:, fc_glob : fc_glob + 1],
                            scale=1.0 / WSCALE,
                        )
                        nc.vector.tensor_tensor(
                            out=gTl[:, mf, js], in0=gq[:], in1=gTh[:, mf, js],
                            op=mybir.AluOpType.subtract,
                        )
                for grp in ((0,), (1,), (2,), (3,), (4,), (5,), (6,), (7,)):
                    pss = {}
                    for tci in grp:
                        psm = ps_mlp.tile(
                            [P, C], F32, tag="psmlp", name=f"psm_{half}_{tci}"
                        )
                        pss[tci] = psm
                    for tci in grp:
                        tsl = slice(tci * P, (tci + 1) * P)
                        m = 0
                        for gt, wt in (
                            (gTh, wmht), (gTl, wmht), (gTh, wmlt),
                        ):
                            for kp in range(NFH // 2):
                                nc.tensor.matmul(
                                    pss[tci][:, 0:512],
                                    gt[:, 2 * kp : 2 * kp + 2, tsl],
                                    wt[:, 2 * kp : 2 * kp + 2, 0:512],
                                    start=(m == 0),
                                    stop=(m == 17),
                                    perf_mode=DR,
                                )
                                nc.tensor.matmul(
                                    pss[tci][:, 512:768],
                                    gt[:, 2 * kp : 2 * kp + 2, tsl],
                                    wt[:, 2 * kp : 2 * kp + 2, 512:768],
                                    start=(m == 0),
                                    stop=(m == 17),
                                    perf_mode=DR,
                                )
                                m += 1
                    for tci in grp:
                        # fused (psum * 1/16) + x1 on DVE undoes the wm x16
                        # pre-scale while draining PSUM
                        nc.vector.scalar_tensor_tensor(
                            out=x1[tci][:], in0=pss[tci][:],
                            scalar=1.0 / WSCALE, in1=x1[tci][:],
                            op0=mybir.AluOpType.mult,
                            op1=mybir.AluOpType.add,
                        )
                        if half == 0:
                            # bm on DVE: the MLP window is PE-bound, DVE idle
                            nc.vector.tensor_add(
                                out=x1[tci][:], in0=x1[tci][:], in1=bm_b[:]
                            )
                        else:
                            nc.sync.dma_start(yr[:, tci, :], x1[tci][:])

        wm_ctx.close()
        mlp_ps_ctx.close()
        wpp_ctx.close()
        ctp_ctx.close()
        mlpw_ctx.close()
        xp_ctx.close()

    nc.compile()
    return nc


# ---------------------------------------------------------------------------
# host wrapper
# ---------------------------------------------------------------------------

_module_cache: dict = {}
_module_lock = threading.Lock()


def _get_module(dbg: bool = False) -> bass.Bass:
    with _module_lock:
        if dbg not in _module_cache:
            _module_cache[dbg] = build_module(dbg)
        return _module_cache[dbg]


def _fold_inputs(
    x, ln1_scale, ln1_bias, w_qkv, b_qkv, w_attn_proj, b_attn_proj,
    ln2_scale, ln2_bias, w_fc, b_fc, w_mlp_proj, b_mlp_proj,
):
    import ml_dtypes

    f32 = np.float32
    bf16 = ml_dtypes.bfloat16
    fp8 = ml_dtypes.float8_e4m3
    w1 = (ln1_scale[:, None].astype(np.float64) * w_qkv.astype(np.float64)).astype(f32)
    b1 = (b_qkv.astype(np.float64) + ln1_bias.astype(np.float64) @ w_qkv.astype(np.float64)).astype(f32)
    w2 = (ln2_scale[:, None].astype(np.float64) * w_fc.astype(np.float64)).astype(f32)
    b2 = (b_fc.astype(np.float64) + ln2_bias.astype(np.float64) @ w_fc.astype(np.float64)).astype(f32)
    def hilo(w):
        # x16 pre-scale keeps the residual (lo) part of these ~N(0, 1/sqrt
        # (fan_in)) weights clear of the fp8e4 denormal floor
        ws = w.astype(np.float64) * WSCALE
        hi = ws.astype(f32).astype(fp8)
        lo = (ws - hi.astype(np.float64)).astype(f32).astype(fp8)
        return np.ascontiguousarray(hi), np.ascontiguousarray(lo)

    w2h, w2l = hilo(w2)
    wmh, wml = hilo(w_mlp_proj.astype(f32))
    w1h, w1l = hilo(w1)
    shared = {
        "w1h": w1h,
        "w1l": w1l,
        "b1": np.ascontiguousarray(b1 * np.float32(WSCALE)),
        "b1v": np.ascontiguousarray(
            (b1[2 * C : 3 * C] * np.float32(WSCALE)).astype(bf16)
        ),
        "wpb": np.ascontiguousarray(
            (w_attn_proj.astype(f32) / np.float32(WSCALE)).astype(bf16)
        ),
        "bp": np.ascontiguousarray(b_attn_proj.astype(f32)),
        "w2h": w2h,
        "w2l": w2l,
        "b2": np.ascontiguousarray(b2),
        "wmh": wmh,
        "wml": wml,
        "bm": np.ascontiguousarray(b_mlp_proj.astype(f32)),
    }
    return [
        {"x": np.ascontiguousarray(x[b].astype(f32).astype(bf16)), **shared} for b in range(B)
    ]


def run(inputs: dict, dbg: bool = False, **spmd_kwargs):
    """Run on 8 cores; returns BassKernelResults."""
    args = {k: np.asarray(v) for k, v in inputs.items()}
    in_maps = _fold_inputs(
        args["x"], args["ln1_scale"], args["ln1_bias"], args["w_qkv"],
        args["b_qkv"], args["w_attn_proj"], args["b_attn_proj"],
        args["ln2_scale"], args["ln2_bias"], args["w_fc"], args["b_fc"],
        args["w_mlp_proj"], args["b_mlp_proj"],
    )
    nc = _get_module(dbg)
    res = run_bass_kernel_spmd(nc, in_maps, core_ids=list(range(B)), **spmd_kwargs)
    return res


def kernel(**inputs) -> np.ndarray:
    res = run(inputs)
    return np.stack([res.results[b]["y"] for b in range(B)], axis=0).astype(
        np.float32
    )


if __name__ == "__main__":
    build_module(dbg=False)
    print("module built OK")
